# revision 20
# baseline (speedup 1.0000x reference)
"""Batched Procrustes-alignment loss on 8 Trainium2 NeuronCores.

Data-parallel over batch (B=262144 -> 32768/core), laid out as [128
partitions, F=256] planes (one scalar per batch element per plane).

v2 pipeline (per core, per For_i iteration):
  DMA raw [P, 51*SUB] f32 sub-chunks; Act de-interleaves+casts to bf16
  component planes [P, 3*JF] (PC/TC after in-place centering). DVE fused-3
  bf16 j-trees give means; fused-3 in-place centered subtract.
  Squares via Act into small ping-pong scratch; folds -> P2/T2 bf16;
  Act sqrt -> sp2st2 (work slot W1); fused-2 j-tree -> pn/tn (PSUM);
  s = tn/(pn+eps) early; d2 = s^2*P2 + T2 seeded before the SVD tail.
  H phase: per r-group one fused-3 product TC_r*PC into work slot W2 and
  a fused-3 bf16 j-tree -> H[r*3+c] = H_{c,r} (f32).
  Closed-form 3x3 eigensolver on A = H^T H (A6 in PSUM, trig eigenvalues,
  eigvecs via cross-of-rows, u_i = H v_i * (-2s/sigma_i), u2 = cross/-2s);
  G' plane (r*3+c) = sum_i u_i[c] v_i[r] (-2s folded), assembled in bf16
  carved out of W1.
  e-phase (no stored O): E_acc[r] = sum_c PC_c * G'_{c,r} (bcast over j),
  then Et = E_acc * TC, folded into d2. dist = sqrt(max(d2,0)); j-tree
  sum; acc += dsum. Host sums [P,1] partials in float64.
"""
import numpy as np
import concourse.bass as bass
import concourse.mybir as mybir
import concourse.tile as tile
from concourse import bacc
from concourse.bass_utils import run_bass_kernel_spmd

AF = mybir.ActivationFunctionType
OP = mybir.AluOpType
AX = mybir.AxisListType
f32 = mybir.dt.float32
bf16 = mybir.dt.bfloat16

B, J, C = 262144, 17, 3
JC = J * C
NCORES = 8
BC = B // NCORES
P = 128
F = 256
JF = J * F
SUB = 32
NSUB = F // SUB
EPS = 1e-8
TINY = 1e-20

# engine assignment knobs ("v" = DVE vector, "g" = gpsimd Pool, "s" = scalar/Act)
KNOBS = dict(
    deint=["s"] * 12 + ["g"] * 4,          # per (tensor*NSUB + sub)
    meantree=["v", "v"],      # per tensor
    center=["v", "v"],        # per tensor
    p2fold=["g", "g", "g", "g"],   # P2+=sq1, P2+=sq2, T2+=sq1, T2+=sq2
    pntree="v",
    d2seed=["v", "g"],        # d2 = P2*s2, d2 += T2
    oprod=["v", "v", "v"],    # per r
    htree=["v", "v", "v"],    # per r
    eprod=["v", "v", "v"],    # per c
    eacc=["v", "v"],
    emul="v",
    wd2=["v", "v", "v"],      # three JF folds into d2
    d2max="v",
    dsum="v",
)


def _ap(t, off, dims):
    a = t[:]
    return bass.AP(a.tensor, a.offset + off, [a.ap[0]] + dims)


def _pl(t, off, n):
    return _ap(t, off, [[1, n]])


def build_nc(iters=1, knobs=None, stop=99, tap=None):
    kn = dict(KNOBS)
    if knobs:
        kn.update(knobs)
    TAP_SHAPES = dict(means=6 * F, PC=3 * JF, TC=3 * JF, P2T2=2 * JF,
                      pntn=2 * F, H=9 * F, G=9 * F, d2=JF)

    nc = bacc.Bacc("TRN2", target_bir_lowering=False)
    pred_d = nc.dram_tensor("pred", [BC, JC], f32, kind="ExternalInput")
    targ_d = nc.dram_tensor("target", [BC, JC], f32, kind="ExternalInput")
    out_d = nc.dram_tensor("partial", [P, 1], f32, kind="ExternalOutput")
    dbg_d = (nc.dram_tensor("dbg", [P, TAP_SHAPES[tap]],
                            f32 if tap in ("H", "pntn") else bf16,
                            kind="ExternalOutput") if tap else None)

    def E(key, i=None):
        v = kn[key] if i is None else kn[key][i]
        return {"v": nc.vector, "g": nc.gpsimd, "s": nc.scalar}[v]

    with tile.TileContext(nc) as tc:
        with (
            tc.tile_pool(name="persist", bufs=1) as persist,
            tc.tile_pool(name="rawp", bufs=1) as rawp,
            tc.tile_pool(name="pctc", bufs=1) as pctcp,
            tc.tile_pool(name="work", bufs=1) as work,
            tc.tile_pool(name="hp", bufs=1) as hp,
            tc.tile_pool(name="late", bufs=1) as late,
            tc.tile_pool(name="thinE", bufs=1) as thinE,
            tc.tile_pool(name="psth", bufs=1, space="PSUM") as psth,
        ):
            acc = persist.tile([P, F], f32, tag="acc", name="acc")
            b2p3 = persist.tile([P, 1], f32, tag="b2p3", name="b2p3")
            b4p3 = persist.tile([P, 1], f32, tag="b4p3", name="b4p3")
            nc.gpsimd.memset(acc[:], 0.0)
            nc.gpsimd.memset(b2p3[:], 2.0943951023931953)  # 2pi/3
            nc.gpsimd.memset(b4p3[:], 1.0471975511965976)  # pi/3

            def thinE_t():
                return thinE.tile([P, F], f32, tag="te", name="te", bufs=12)

            def named(tg):
                return thinE.tile([P, F], f32, tag="An", name=tg, bufs=12)

            _ps = {"n": 0, "banks": []}

            def psum_t(tg):
                i = _ps["n"]
                _ps["n"] += 1
                assert i < 16
                if i % 2 == 0:
                    _ps["banks"].append(
                        psth.tile([P, 2 * F], f32, tag=f"pb{i // 2}",
                                  name=f"pb{i // 2}"))
                blk = _ps["banks"][i // 2]
                off = (i % 2) * F

                class _T:
                    def __getitem__(self, _):
                        return _pl(blk, off, F)
                return _T()

            def body():
                _ps["n"] = 0
                _ps["banks"] = []
                # --------- persistent-ish tiles for this iteration
                PC = pctcp.tile([P, 3 * JF], bf16, tag="PC", name="PC")
                TC = pctcp.tile([P, 3 * JF], bf16, tag="TC", name="TC")
                means = pctcp.tile([P, 6 * F], bf16, tag="mn", name="means")
                ht = hp.tile([P, 3 * 8 * F], bf16, tag="ht", name="ht")
                H = hp.tile([P, 9 * F], f32, tag="H", name="H")
                d2 = late.tile([P, JF], bf16, tag="d2", name="d2")
                P2T2 = late.tile([P, 2 * JF], bf16, tag="p2", name="P2T2")
                Gp = late.tile([P, 9 * F], bf16, tag="G", name="Gp")

                def g3(t, off, inner=F):
                    return _ap(t, off, [[JF, 3], [F, J], [1, inner]])

                # --------- load + Act de-interleave/cast + mean + center
                for ti, (dram, ctr) in enumerate(((pred_d, PC), (targ_d, TC))):
                    for s in range(NSUB):
                        raw = rawp.tile([P, JC * SUB], f32, tag="raw",
                                        name="raw", bufs=2)
                        off = (s * SUB) * JC
                        nc.sync.dma_start(
                            raw[:], bass.AP(dram[:].tensor, off,
                                            [[F * JC, P], [1, JC * SUB]]))
                        # ctr[c][j][s*SUB+u] = raw[u*JC + j*3 + c]
                        de = E("deint", ti * NSUB + s)
                        dd_ = (_ap(ctr, s * SUB, [[JF, 3], [F, J], [1, SUB]]),
                               _ap(raw, 0, [[1, 3], [3, J], [JC, SUB]]))
                        if de is nc.scalar:
                            de.activation(dd_[0], dd_[1], AF.Copy)
                        else:
                            de.tensor_copy(dd_[0], dd_[1])
                    mn = _ap(means, ti * 3 * F, [[F, 3], [1, F]])
                    et = E("meantree", ti)
                    et.tensor_tensor(
                        _ap(ht, 0, [[8 * F, 3], [F, 8], [1, F]]),
                        _ap(ctr, 0, [[JF, 3], [F, 8], [1, F]]),
                        _ap(ctr, 8 * F, [[JF, 3], [F, 8], [1, F]]), OP.add)
                    et.tensor_tensor(
                        _ap(ht, 0, [[8 * F, 3], [F, 4], [1, F]]),
                        _ap(ht, 0, [[8 * F, 3], [F, 4], [1, F]]),
                        _ap(ht, 4 * F, [[8 * F, 3], [F, 4], [1, F]]), OP.add)
                    et.tensor_tensor(
                        _ap(ht, 0, [[8 * F, 3], [F, 2], [1, F]]),
                        _ap(ht, 0, [[8 * F, 3], [F, 2], [1, F]]),
                        _ap(ht, 2 * F, [[8 * F, 3], [F, 2], [1, F]]), OP.add)
                    et.tensor_tensor(
                        _ap(ht, 0, [[8 * F, 3], [1, F]]),
                        _ap(ht, 0, [[8 * F, 3], [1, F]]),
                        _ap(ht, F, [[8 * F, 3], [1, F]]), OP.add)
                    et.tensor_tensor(
                        mn,
                        _ap(ht, 0, [[8 * F, 3], [1, F]]),
                        _ap(ctr, 16 * F, [[JF, 3], [1, F]]), OP.add)
                    nc.vector.tensor_scalar_mul(mn, mn, 1.0 / J)
                    E("center", ti).tensor_tensor(
                        g3(ctr, 0), g3(ctr, 0),
                        _ap(means, ti * 3 * F, [[F, 3], [0, J], [1, F]]),
                        OP.subtract)

                if tap == "means":
                    nc.sync.dma_start(dbg_d[:], means[:])
                if tap == "PC":
                    nc.sync.dma_start(dbg_d[:], PC[:])
                if tap == "TC":
                    nc.sync.dma_start(dbg_d[:], TC[:])
                if stop <= 0:
                    return

                # --------- H phase: per r-group product + fused-3 bf16 j-tree
                # Op plane (r*3+c) = TC_r * PC_c ; H plane (r*3+c) = H_{c,r}
                W2 = work.tile([P, 3 * JF], bf16, tag="W2", name="W2a")
                for r in range(3):
                    E("oprod", r).tensor_tensor(
                        g3(W2, 0),
                        _ap(TC, r * JF, [[0, 3], [F, J], [1, F]]),
                        g3(PC, 0), OP.mult)
                    et = E("htree", r)
                    et.tensor_tensor(
                        _ap(ht, 0, [[8 * F, 3], [F, 8], [1, F]]),
                        _ap(W2, 0, [[JF, 3], [F, 8], [1, F]]),
                        _ap(W2, 8 * F, [[JF, 3], [F, 8], [1, F]]), OP.add)
                    et.tensor_tensor(
                        _ap(ht, 0, [[8 * F, 3], [F, 4], [1, F]]),
                        _ap(ht, 0, [[8 * F, 3], [F, 4], [1, F]]),
                        _ap(ht, 4 * F, [[8 * F, 3], [F, 4], [1, F]]), OP.add)
                    et.tensor_tensor(
                        _ap(ht, 0, [[8 * F, 3], [F, 2], [1, F]]),
                        _ap(ht, 0, [[8 * F, 3], [F, 2], [1, F]]),
                        _ap(ht, 2 * F, [[8 * F, 3], [F, 2], [1, F]]), OP.add)
                    et.tensor_tensor(
                        _ap(ht, 0, [[8 * F, 3], [1, F]]),
                        _ap(ht, 0, [[8 * F, 3], [1, F]]),
                        _ap(ht, F, [[8 * F, 3], [1, F]]), OP.add)
                    et.tensor_tensor(
                        _ap(H, r * 3 * F, [[F, 3], [1, F]]),
                        _ap(ht, 0, [[8 * F, 3], [1, F]]),
                        _ap(W2, 16 * F, [[JF, 3], [1, F]]), OP.add)

                if tap == "H":
                    nc.sync.dma_start(dbg_d[:], H[:])

                # --------- squares -> P2/T2 (Act into ht/d2 ping-pong scratch)
                P2 = _pl(P2T2, 0, JF)
                T2 = _pl(P2T2, JF, JF)
                sqs = late.tile([P, JF], bf16, tag="sqs", name="sqs")
                sqh = sqs[:]
                sqd = d2[:]              # d2 free until seed; dist(k-1) done
                # by the time squares(k) run in steady state
                for ti, (ctr, dst) in enumerate(((PC, P2), (TC, T2))):
                    nc.scalar.activation(dst, _pl(ctr, 0, JF), AF.Square)
                    nc.scalar.activation(sqh, _pl(ctr, JF, JF), AF.Square)
                    nc.scalar.activation(sqd, _pl(ctr, 2 * JF, JF), AF.Square)
                    E("p2fold", ti * 2).tensor_tensor(dst, dst, sqh, OP.add)
                    E("p2fold", ti * 2 + 1).tensor_tensor(dst, dst, sqd, OP.add)

                # sqrt -> sp2st2 in work slot W1; fused-2 j-tree -> pn/tn
                W1 = work.tile([P, 3 * JF], bf16, tag="W1", name="W1a")
                sp2st2 = _pl(W1, 0, 2 * JF)
                nc.scalar.activation(sp2st2, P2T2[:], AF.Sqrt)
                pntn = psth.tile([P, 2 * F], f32, tag="pntn", name="pntn")
                et = E("pntree")
                et.tensor_tensor(
                    _ap(ht, 0, [[8 * F, 2], [F, 8], [1, F]]),
                    _ap(W1, 0, [[JF, 2], [F, 8], [1, F]]),
                    _ap(W1, 8 * F, [[JF, 2], [F, 8], [1, F]]), OP.add)
                et.tensor_tensor(
                    _ap(ht, 0, [[8 * F, 2], [F, 4], [1, F]]),
                    _ap(ht, 0, [[8 * F, 2], [F, 4], [1, F]]),
                    _ap(ht, 4 * F, [[8 * F, 2], [F, 4], [1, F]]), OP.add)
                et.tensor_tensor(
                    _ap(ht, 0, [[8 * F, 2], [F, 2], [1, F]]),
                    _ap(ht, 0, [[8 * F, 2], [F, 2], [1, F]]),
                    _ap(ht, 2 * F, [[8 * F, 2], [F, 2], [1, F]]), OP.add)
                et.tensor_tensor(
                    _ap(ht, 0, [[8 * F, 2], [1, F]]),
                    _ap(ht, 0, [[8 * F, 2], [1, F]]),
                    _ap(ht, F, [[8 * F, 2], [1, F]]), OP.add)
                et.tensor_tensor(
                    _ap(pntn, 0, [[F, 2], [1, F]]),
                    _ap(ht, 0, [[8 * F, 2], [1, F]]),
                    _ap(W1, 16 * F, [[JF, 2], [1, F]]), OP.add)
                pn = _pl(pntn, 0, F)
                tn = _pl(pntn, F, F)
                if tap == "P2T2":
                    nc.sync.dma_start(dbg_d[:], P2T2[:])
                if tap == "pntn":
                    pncp = late.tile([P, 2 * F], f32, tag="pncp", name="pncp")
                    nc.vector.tensor_copy(pncp[:], pntn[:])
                    nc.sync.dma_start(dbg_d[:], pncp[:])

                # s = tn/(pn+eps); s2 bf16 (SBUF); seed d2 = s^2*P2 + T2
                sS = named("sS")
                nc.vector.tensor_scalar_add(sS[:], pn, EPS)
                nc.vector.reciprocal_approx_fast(sS[:], sS[:])
                nc.vector.tensor_tensor(sS[:], sS[:], tn, OP.mult)
                s2b = late.tile([P, F], bf16, tag="s2b", name="s2b")
                nc.vector.tensor_tensor(s2b[:], sS[:], sS[:], OP.mult)
                E("d2seed", 0).tensor_tensor(
                    d2[:], P2, _ap(s2b, 0, [[0, J], [1, F]]), OP.mult)
                E("d2seed", 1).tensor_tensor(d2[:], d2[:], T2, OP.add)

                if stop <= 1:
                    return

                def Hp(a, cc):
                    # H_{cc,a} (pred comp cc, targ comp a) = plane (a*3+cc)
                    return _pl(H, (a * 3 + cc) * F, F)

                if stop <= 2:
                    return

                # --------- A = H^T H (6 upper entries) in PSUM, f32
                A6 = {}
                for (a, b) in ((0, 0), (0, 1), (0, 2), (1, 1), (1, 2), (2, 2)):
                    t1 = thinE_t()
                    nc.vector.tensor_tensor(t1[:], Hp(a, 0), Hp(b, 0), OP.mult)
                    t2 = thinE_t()
                    nc.vector.tensor_tensor(t2[:], Hp(a, 1), Hp(b, 1), OP.mult)
                    nc.vector.tensor_tensor(t1[:], t1[:], t2[:], OP.add)
                    t3 = thinE_t()
                    nc.vector.tensor_tensor(t3[:], Hp(a, 2), Hp(b, 2), OP.mult)
                    At = named(f"A{a}{b}")
                    nc.vector.tensor_tensor(At[:], t1[:], t3[:], OP.add)
                    A6[(a, b)] = At
                a00, a01, a02 = A6[(0, 0)], A6[(0, 1)], A6[(0, 2)]
                a11, a12, a22 = A6[(1, 1)], A6[(1, 2)], A6[(2, 2)]

                # --------- eigenvalues (closed form, f32)
                q3 = thinE_t()
                nc.vector.tensor_tensor(q3[:], a00[:], a11[:], OP.add)
                nc.vector.tensor_tensor(q3[:], q3[:], a22[:], OP.add)
                m01, g0, g1 = named("m01"), named("g0"), named("g1")
                g2 = named("g2")
                nc.vector.tensor_tensor(m01[:], a01[:], a01[:], OP.mult)
                nc.vector.tensor_tensor(g0[:], a01[:], a12[:], OP.mult)
                nc.vector.tensor_tensor(g1[:], a01[:], a02[:], OP.mult)
                nc.vector.tensor_tensor(g2[:], a02[:], a12[:], OP.mult)
                m02 = thinE_t()
                nc.vector.tensor_tensor(m02[:], a02[:], a02[:], OP.mult)
                m12 = thinE_t()
                nc.vector.tensor_tensor(m12[:], a12[:], a12[:], OP.mult)
                p1 = thinE_t()
                nc.vector.tensor_tensor(p1[:], m01[:], m02[:], OP.add)
                nc.vector.tensor_tensor(p1[:], p1[:], m12[:], OP.add)
                q = named("q")
                nc.vector.tensor_scalar_mul(q[:], q3[:], 1.0 / 3)
                b00, b11, b22 = thinE_t(), thinE_t(), thinE_t()
                nc.vector.tensor_tensor(b00[:], a00[:], q[:], OP.subtract)
                nc.vector.tensor_tensor(b11[:], a11[:], q[:], OP.subtract)
                nc.vector.tensor_tensor(b22[:], a22[:], q[:], OP.subtract)
                p2s = thinE_t()
                nc.vector.tensor_tensor(p2s[:], b00[:], b00[:], OP.mult)
                tb = thinE_t()
                nc.vector.tensor_tensor(tb[:], b11[:], b11[:], OP.mult)
                nc.vector.tensor_tensor(p2s[:], p2s[:], tb[:], OP.add)
                nc.vector.tensor_tensor(tb[:], b22[:], b22[:], OP.mult)
                nc.vector.tensor_tensor(p2s[:], p2s[:], tb[:], OP.add)
                nc.vector.scalar_tensor_tensor(
                    p2s[:], p1[:], 2.0, p2s[:], OP.mult, OP.add)
                pA = named("pA")
                nc.scalar.activation(pA[:], p2s[:], AF.Sqrt, scale=1.0 / 6)
                c0 = thinE_t()
                nc.vector.tensor_tensor(c0[:], b11[:], b22[:], OP.mult)
                nc.vector.tensor_tensor(c0[:], c0[:], m12[:], OP.subtract)
                c1 = thinE_t()
                nc.vector.tensor_tensor(c1[:], a01[:], b22[:], OP.mult)
                nc.vector.tensor_tensor(c1[:], c1[:], g2[:], OP.subtract)
                c2 = thinE_t()
                nc.vector.tensor_tensor(c2[:], b11[:], a02[:], OP.mult)
                nc.vector.tensor_tensor(c2[:], g0[:], c2[:], OP.subtract)
                detB = thinE_t()
                nc.vector.tensor_tensor(detB[:], b00[:], c0[:], OP.mult)
                tdb = thinE_t()
                nc.vector.tensor_tensor(tdb[:], a01[:], c1[:], OP.mult)
                nc.vector.tensor_tensor(detB[:], detB[:], tdb[:], OP.subtract)
                nc.vector.tensor_tensor(tdb[:], a02[:], c2[:], OP.mult)
                nc.vector.tensor_tensor(detB[:], detB[:], tdb[:], OP.add)
                pinv = thinE_t()
                nc.vector.tensor_scalar_add(pinv[:], pA[:], TINY)
                nc.vector.reciprocal_approx_fast(pinv[:], pinv[:])
                p3 = thinE_t()
                nc.vector.tensor_tensor(p3[:], pinv[:], pinv[:], OP.mult)
                nc.vector.tensor_tensor(p3[:], p3[:], pinv[:], OP.mult)
                rc = thinE_t()
                nc.vector.tensor_tensor(rc[:], detB[:], p3[:], OP.mult)
                nc.vector.tensor_scalar(rc[:], rc[:], 0.5, 1.0, OP.mult, OP.min)
                nc.vector.tensor_scalar_max(rc[:], rc[:], -1.0)
                rr = thinE_t()
                nc.vector.tensor_tensor(rr[:], rc[:], rc[:], OP.mult)
                wA = thinE_t()
                nc.scalar.activation(wA[:], rr[:], AF.Sqrt, bias=1.0, scale=-1.0)
                rat = thinE_t()
                nc.vector.tensor_scalar_add(rat[:], wA[:], 1e-10)
                nc.vector.reciprocal_approx_fast(rat[:], rat[:])
                nc.vector.tensor_tensor(rat[:], rc[:], rat[:], OP.mult)
                a1 = thinE_t()
                nc.vector.tensor_scalar(a1[:], rat[:], 1.0, -1.0, OP.min, OP.max)
                rat2 = thinE_t()
                nc.vector.tensor_tensor(rat2[:], rat[:], rat[:], OP.mult)
                rinv = thinE_t()
                nc.vector.tensor_scalar_add(rinv[:], rat2[:], TINY)
                nc.vector.reciprocal_approx_fast(rinv[:], rinv[:])
                nc.vector.tensor_tensor(rinv[:], rat[:], rinv[:], OP.mult)
                nc.vector.tensor_scalar(rinv[:], rinv[:], 1.0, -1.0, OP.min, OP.max)
                sg = thinE_t()
                nc.vector.tensor_scalar(sg[:], rat[:], 1e10, 1.0, OP.mult, OP.min)
                nc.vector.tensor_scalar_max(sg[:], sg[:], -1.0)
                at1 = thinE_t()
                nc.scalar.activation(at1[:], a1[:], AF.Arctan)
                at2 = thinE_t()
                nc.scalar.activation(at2[:], rinv[:], AF.Arctan)
                atb = thinE_t()
                nc.vector.scalar_tensor_tensor(
                    atb[:], sg[:], 1.5707963267948966, at2[:],
                    OP.mult, OP.subtract)
                m_ = thinE_t()
                nc.vector.tensor_scalar_add(m_[:], rat2[:], -1.0)
                nc.vector.tensor_scalar(m_[:], m_[:], 1e10, 1.0, OP.mult, OP.min)
                nc.vector.tensor_scalar_max(m_[:], m_[:], 0.0)
                atn = thinE_t()
                nc.vector.tensor_tensor(atn[:], atb[:], at1[:], OP.subtract)
                nc.vector.tensor_tensor(atn[:], atn[:], m_[:], OP.mult)
                nc.vector.tensor_tensor(atn[:], atn[:], at1[:], OP.add)
                cs1 = psum_t("cs1")
                nc.scalar.activation(cs1[:], atn[:], AF.Sin,
                                     bias=b2p3[:], scale=-1.0 / 3)
                cs2 = psum_t("cs2")
                nc.scalar.activation(cs2[:], atn[:], AF.Sin,
                                     bias=b4p3[:], scale=-1.0 / 3)
                lam0, lam1 = psum_t("lam0"), psum_t("lam1")
                tp = thinE_t()
                nc.vector.tensor_tensor(tp[:], pA[:], cs1[:], OP.mult)
                nc.vector.scalar_tensor_tensor(
                    lam0[:], tp[:], 2.0, q[:], OP.mult, OP.add)
                lam2 = thinE_t()
                nc.vector.tensor_tensor(tp[:], pA[:], cs2[:], OP.mult)
                nc.vector.scalar_tensor_tensor(
                    lam2[:], tp[:], -2.0, q[:], OP.mult, OP.add)
                nc.vector.scalar_tensor_tensor(
                    lam1[:], q[:], 3.0, lam0[:], OP.mult, OP.subtract)
                nc.vector.tensor_tensor(lam1[:], lam1[:], lam2[:], OP.subtract)

                # --------- W1 carve for bf16 tail (sp2st2 dead after
                # pntree). layout: Hb 9F | vb 9F | ub 6F | u2t 3F | gt 3F |
                # gt2 3F | rsb 2F | invsb F
                W1b = work.tile([P, 3 * JF], bf16, tag="W1", name="W1b")
                invsb = _pl(W1b, 35 * F, F)
                nc.vector.tensor_copy(_pl(W1b, 0, 9 * F), H[:])

                def vbp(i, k):
                    return _pl(W1b, (9 + i * 3 + k) * F, F)

                # --------- eigenvectors v0, v1 (f32 transient -> bf16 vb)
                def eigvec(lam, vbi):
                    vx = thinE_t()
                    vy = thinE_t()
                    vz = thinE_t()
                    b0 = thinE_t()
                    nc.vector.tensor_tensor(b0[:], a00[:], lam[:], OP.subtract)
                    b1 = thinE_t()
                    nc.vector.tensor_tensor(b1[:], a11[:], lam[:], OP.subtract)
                    nc.vector.tensor_tensor(vx[:], a02[:], b1[:], OP.mult)
                    nc.vector.tensor_tensor(vx[:], g0[:], vx[:], OP.subtract)
                    nc.vector.tensor_tensor(vy[:], b0[:], a12[:], OP.mult)
                    nc.vector.tensor_tensor(vy[:], g1[:], vy[:], OP.subtract)
                    nc.vector.tensor_tensor(vz[:], b0[:], b1[:], OP.mult)
                    nc.vector.tensor_tensor(vz[:], vz[:], m01[:], OP.subtract)
                    n2 = thinE_t()
                    nc.vector.tensor_tensor(n2[:], vx[:], vx[:], OP.mult)
                    t2_ = thinE_t()
                    nc.vector.tensor_tensor(t2_[:], vy[:], vy[:], OP.mult)
                    nc.vector.tensor_tensor(n2[:], n2[:], t2_[:], OP.add)
                    nc.vector.tensor_tensor(t2_[:], vz[:], vz[:], OP.mult)
                    nc.vector.tensor_tensor(n2[:], n2[:], t2_[:], OP.add)
                    ns = thinE_t()
                    nc.scalar.activation(ns[:], n2[:], AF.Sqrt)
                    nc.vector.tensor_scalar_add(ns[:], ns[:], TINY)
                    nc.vector.reciprocal_approx_fast(ns[:], ns[:])
                    nc.vector.tensor_tensor(vbp(vbi, 0), vx[:], ns[:], OP.mult)
                    nc.vector.tensor_tensor(vbp(vbi, 1), vy[:], ns[:], OP.mult)
                    nc.vector.tensor_tensor(vbp(vbi, 2), vz[:], ns[:], OP.mult)

                eigvec(lam0, 0)
                eigvec(lam1, 1)

                # v2 = v0 x v1 (bf16)
                cr = ((1, 2), (2, 0), (0, 1))
                for r_ in range(3):
                    i1, i2 = cr[r_]
                    t1b = _pl(W1b, 27 * F, F)
                    t2b = _pl(W1b, 28 * F, F)
                    nc.vector.tensor_tensor(t1b, vbp(0, i1), vbp(1, i2),
                                            OP.mult)
                    nc.vector.tensor_tensor(t2b, vbp(0, i2), vbp(1, i1),
                                            OP.mult)
                    nc.vector.tensor_tensor(vbp(2, r_), t1b, t2b, OP.subtract)

                # --------- rsig_i = -2s/sigma_i (bf16 into rsb)
                for i, lam in enumerate((lam0, lam1)):
                    rl = thinE_t()
                    nc.scalar.activation(rl[:], lam[:], AF.Relu)
                    sg_ = thinE_t()
                    nc.scalar.activation(sg_[:], rl[:], AF.Sqrt)
                    nc.vector.tensor_scalar_add(sg_[:], sg_[:], TINY)
                    nc.vector.reciprocal_approx_fast(sg_[:], sg_[:])
                    nc.vector.scalar_tensor_tensor(
                        _pl(W1b, (33 + i) * F, F), sg_[:], -2.0, sS[:],
                        OP.mult, OP.mult)
                iv_ = thinE_t()
                nc.vector.tensor_scalar_add(iv_[:], sS[:], TINY)
                nc.vector.reciprocal_approx_fast(iv_[:], iv_[:])
                nc.vector.tensor_scalar_mul(iv_[:], iv_[:], -0.5)
                nc.vector.tensor_copy(invsb, iv_[:])

                def HCg(k):
                    # planes (k*3 + r) = H_{r,k}, r=0..2
                    return _ap(W1b, k * 3 * F, [[F, 3], [1, F]])

                def vbc(i, k):
                    return _ap(W1b, (9 + i * 3 + k) * F, [[0, 3], [1, F]])

                # u_i[r] = sum_k H_{r,k} (v_i)_k, scaled by rsig_i
                for i in range(2):
                    udst = _ap(W1b, (18 + i * 3) * F, [[F, 3], [1, F]])
                    nc.vector.tensor_tensor(udst, HCg(0), vbc(i, 0), OP.mult)
                    gta = _ap(W1b, 27 * F, [[F, 3], [1, F]])
                    nc.vector.tensor_tensor(gta, HCg(1), vbc(i, 1), OP.mult)
                    nc.vector.tensor_tensor(udst, udst, gta, OP.add)
                    nc.vector.tensor_tensor(gta, HCg(2), vbc(i, 2), OP.mult)
                    nc.vector.tensor_tensor(udst, udst, gta, OP.add)
                    nc.vector.tensor_tensor(
                        udst, udst, _ap(W1b, (33 + i) * F, [[0, 3], [1, F]]),
                        OP.mult)

                def up(ui, r_):
                    return _pl(W1b, (18 + ui * 3 + r_) * F, F)

                # u2 = cross(u0, u1) * (-0.5/s)
                for r_ in range(3):
                    i1, i2 = cr[r_]
                    t1b = _pl(W1b, 27 * F, F)
                    t2b = _pl(W1b, 28 * F, F)
                    nc.vector.tensor_tensor(t1b, up(0, i1), up(1, i2), OP.mult)
                    nc.vector.tensor_tensor(t2b, up(0, i2), up(1, i1), OP.mult)
                    nc.vector.tensor_tensor(t1b, t1b, t2b, OP.subtract)
                    nc.vector.tensor_tensor(
                        _pl(W1b, (24 + r_) * F, F), t1b, invsb, OP.mult)

                # --------- G' plane (r*3+c) = sum_i u_i[c] * (v_i)_r
                def ug(i):
                    base = (18 + i * 3) * F if i < 2 else 24 * F
                    return _ap(W1b, base, [[F, 3], [1, F]])

                gta = _ap(W1b, 27 * F, [[F, 3], [1, F]])
                gtb = _ap(W1b, 30 * F, [[F, 3], [1, F]])
                for r_ in range(3):
                    # G' plane (r*3+c) = sum_i u_i[c] * v_r[i]  (V^T quirk of
                    # the reference: R = Vh @ Ut, so the contraction pairs
                    # u_i with the i-th COMPONENT of v_r)
                    Grg = _ap(Gp, r_ * 3 * F, [[F, 3], [1, F]])
                    nc.vector.tensor_tensor(gta, ug(0), vbc(r_, 0), OP.mult)
                    nc.vector.tensor_tensor(gtb, ug(1), vbc(r_, 1), OP.mult)
                    nc.vector.tensor_tensor(gta, gta, gtb, OP.add)
                    nc.vector.tensor_tensor(gtb, ug(2), vbc(r_, 2), OP.mult)
                    nc.vector.tensor_tensor(Grg, gta, gtb, OP.add)

                if tap == "G":
                    nc.sync.dma_start(dbg_d[:], Gp[:])
                if stop <= 3:
                    return

                # --------- e-phase: E_acc[r] = sum_c PC_c * G'_{c,r}
                W2b = work.tile([P, 3 * JF], bf16, tag="W2", name="W2b")
                Ea = g3(W2b, 0)
                W1c = work.tile([P, 3 * JF], bf16, tag="W1", name="W1c")
                Et = g3(W1c, 0)

                def gpc(c):
                    # G' planes (r*3+c) for r=0..2: offset c*F, stride 3F
                    return _ap(Gp, c * F, [[3 * F, 3], [0, J], [1, F]])

                E("eprod", 0).tensor_tensor(
                    Ea, _ap(PC, 0, [[0, 3], [F, J], [1, F]]), gpc(0), OP.mult)
                E("eprod", 1).tensor_tensor(
                    Et, _ap(PC, JF, [[0, 3], [F, J], [1, F]]), gpc(1), OP.mult)
                E("eacc", 0).tensor_tensor(Ea, Ea, Et, OP.add)
                E("eprod", 2).tensor_tensor(
                    Et, _ap(PC, 2 * JF, [[0, 3], [F, J], [1, F]]), gpc(2),
                    OP.mult)
                E("eacc", 1).tensor_tensor(Ea, Ea, Et, OP.add)
                # Et = E_acc * TC (aligned r-planes); fold into d2
                E("emul").tensor_tensor(Et, Ea, g3(TC, 0), OP.mult)
                for c in range(3):
                    E("wd2", c).tensor_tensor(
                        d2[:], d2[:], _pl(W1c, c * JF, JF), OP.add)

                if tap == "d2":
                    nc.sync.dma_start(dbg_d[:], d2[:])
                # --------- dist = sqrt(max(d2,0)); j-tree; accumulate
                E("d2max").tensor_scalar_max(d2[:], d2[:], 0.0)
                dr = sqs[:]
                nc.scalar.activation(dr, d2[:], AF.Sqrt)
                dh = Gp  # dist-tree scratch aliases G (dead after e-prods)
                et = E("dsum")
                et.tensor_tensor(
                    _ap(dh, 0, [[F, 8], [1, F]]),
                    _ap(sqs, 0, [[F, 8], [1, F]]),
                    _ap(sqs, 8 * F, [[F, 8], [1, F]]), OP.add)
                et.tensor_tensor(
                    _ap(dh, 0, [[F, 4], [1, F]]),
                    _ap(dh, 0, [[F, 4], [1, F]]),
                    _ap(dh, 4 * F, [[F, 4], [1, F]]), OP.add)
                et.tensor_tensor(
                    _ap(dh, 0, [[F, 2], [1, F]]),
                    _ap(dh, 0, [[F, 2], [1, F]]),
                    _ap(dh, 2 * F, [[F, 2], [1, F]]), OP.add)
                et.tensor_tensor(
                    _pl(dh, 0, F), _pl(dh, 0, F), _pl(dh, F, F), OP.add)
                et.tensor_tensor(
                    _pl(dh, 0, F), _pl(dh, 0, F), _pl(sqs, 16 * F, F), OP.add)
                nc.vector.tensor_tensor(acc[:], acc[:], _pl(dh, 0, F), OP.add)

            if iters == 1:
                body()
            elif iters % 2 == 0:
                # two bodies per trip: the all-engine barrier For_i inserts
                # per trip then costs half, and body B's front overlaps body
                # A's tail through the tag-ring dependencies.
                with tc.For_i(0, iters // 2, 1):
                    body()
                    body()
            else:
                with tc.For_i(0, iters, 1):
                    body()

            accs = persist.tile([P, 1], f32, tag="accs", name="accs")
            nc.vector.tensor_reduce(accs[:], acc[:], axis=AX.X, op=OP.add)
            nc.sync.dma_start(out_d[:], accs[:])

    nc.compile()
    return nc


def build_tapped(tap):
    nc = build_nc(iters=1, tap=tap)
    return nc, (lambda x: x)


_nc_cache = None


def get_nc():
    global _nc_cache
    if _nc_cache is None:
        _nc_cache = build_nc()
    return _nc_cache


def run(nc, pred, target, trace=False, **kw):
    pred2 = np.ascontiguousarray(np.asarray(pred), np.float32).reshape(B, JC)
    targ2 = np.ascontiguousarray(np.asarray(target), np.float32).reshape(B, JC)
    in_maps = [
        {"pred": pred2[c * BC:(c + 1) * BC], "target": targ2[c * BC:(c + 1) * BC]}
        for c in range(NCORES)
    ]
    res = run_bass_kernel_spmd(nc, in_maps, list(range(NCORES)), trace=trace, **kw)
    total = sum(r["partial"].astype(np.float64).sum() for r in res.results)
    loss = np.float32(total / (B * J))
    return loss, res


def kernel(pred, target):
    loss, _ = run(get_nc(), pred, target)
    return loss


# revision 26
# speedup vs baseline: 1.3704x; 1.3704x over previous
"""Batched Procrustes-alignment loss on 8 Trainium2 NeuronCores.

Data-parallel over batch (B=262144 -> 32768/core), laid out as [128
partitions, F=256] planes (one scalar per batch element per plane).

v2 pipeline (per core, per For_i iteration):
  DMA raw [P, 51*SUB] f32 sub-chunks; Act de-interleaves+casts to bf16
  component planes [P, 3*JF] (PC/TC after in-place centering). DVE fused-3
  bf16 j-trees give means; fused-3 in-place centered subtract.
  Squares via Act into small ping-pong scratch; folds -> P2/T2 bf16;
  Act sqrt -> sp2st2 (work slot W1); fused-2 j-tree -> pn/tn (PSUM);
  s = tn/(pn+eps) early; d2 = s^2*P2 + T2 seeded before the SVD tail.
  H phase: per r-group one fused-3 product TC_r*PC into work slot W2 and
  a fused-3 bf16 j-tree -> H[r*3+c] = H_{c,r} (f32).
  Closed-form 3x3 eigensolver on A = H^T H (A6 in PSUM, trig eigenvalues,
  eigvecs via cross-of-rows, u_i = H v_i * (-2s/sigma_i), u2 = cross/-2s);
  G' plane (r*3+c) = sum_i u_i[c] v_i[r] (-2s folded), assembled in bf16
  carved out of W1.
  e-phase (no stored O): E_acc[r] = sum_c PC_c * G'_{c,r} (bcast over j),
  then Et = E_acc * TC, folded into d2. dist = sqrt(max(d2,0)); j-tree
  sum; acc += dsum. Host sums [P,1] partials in float64.
"""
import numpy as np
import concourse.bass as bass
import concourse.mybir as mybir
import concourse.tile as tile
from concourse import bacc
from concourse.bass_utils import run_bass_kernel_spmd

AF = mybir.ActivationFunctionType
OP = mybir.AluOpType
AX = mybir.AxisListType
f32 = mybir.dt.float32
bf16 = mybir.dt.bfloat16

B, J, C = 262144, 17, 3
JC = J * C
NCORES = 8
BC = B // NCORES
P = 128
F = 256
JF = J * F
SUB = 32
NSUB = F // SUB
EPS = 1e-8
TINY = 1e-20

# engine assignment knobs ("v" = DVE vector, "g" = gpsimd Pool, "s" = scalar/Act)
KNOBS = dict(
    deint=["s"] * 16,          # per (tensor*NSUB + sub)
    meantree=["v", "v"],      # per tensor
    center=["v", "v"],        # per tensor
    p2fold=["v", "v", "v", "v"],   # P2+=sq1, P2+=sq2, T2+=sq1, T2+=sq2
    pntree="v",
    d2seed=["v", "v"],        # d2 = P2*s2, d2 += T2
    oprod=["v", "v", "v"],    # per r
    htree=["v", "v", "v"],    # per r
    eprod=["v", "v", "v"],    # per c
    eacc=["v", "v"],
    emul="v",
    wd2=["v", "v", "v"],      # three JF folds into d2
    d2max="v",
    dsum="v",
)


def _ap(t, off, dims):
    a = t[:]
    return bass.AP(a.tensor, a.offset + off, [a.ap[0]] + dims)


def _pl(t, off, n):
    return _ap(t, off, [[1, n]])


def build_nc(iters=1, knobs=None, stop=99, tap=None, unroll=4):
    kn = dict(KNOBS)
    if knobs:
        kn.update(knobs)
    TAP_SHAPES = dict(means=6 * F, PC=3 * JF, TC=3 * JF, P2T2=2 * JF,
                      pntn=2 * F, H=9 * F, G=9 * F, d2=JF)

    nc = bacc.Bacc("TRN2", target_bir_lowering=False)
    pred_d = nc.dram_tensor("pred", [BC, JC], f32, kind="ExternalInput")
    targ_d = nc.dram_tensor("target", [BC, JC], f32, kind="ExternalInput")
    out_d = nc.dram_tensor("partial", [P, 1], f32, kind="ExternalOutput")
    dbg_d = (nc.dram_tensor("dbg", [P, TAP_SHAPES[tap]],
                            f32 if tap in ("H", "pntn") else bf16,
                            kind="ExternalOutput") if tap else None)

    def E(key, i=None):
        v = kn[key] if i is None else kn[key][i]
        return {"v": nc.vector, "g": nc.gpsimd, "s": nc.scalar}[v]

    with tile.TileContext(nc) as tc:
        with (
            tc.tile_pool(name="persist", bufs=1) as persist,
            tc.tile_pool(name="rawp", bufs=1) as rawp,
            tc.tile_pool(name="pctc", bufs=1) as pctcp,
            tc.tile_pool(name="work", bufs=1) as work,
            tc.tile_pool(name="hp", bufs=1) as hp,
            tc.tile_pool(name="late", bufs=1) as late,
            tc.tile_pool(name="thinE", bufs=1) as thinE,
            tc.tile_pool(name="psth", bufs=1, space="PSUM") as psth,
        ):
            acc = persist.tile([P, F], f32, tag="acc", name="acc")
            b2p3 = persist.tile([P, 1], f32, tag="b2p3", name="b2p3")
            b4p3 = persist.tile([P, 1], f32, tag="b4p3", name="b4p3")
            nc.gpsimd.memset(acc[:], 0.0)
            nc.gpsimd.memset(b2p3[:], 2.0943951023931953)  # 2pi/3
            nc.gpsimd.memset(b4p3[:], 1.0471975511965976)  # pi/3

            def thinE_t():
                return thinE.tile([P, F], f32, tag="te", name="te", bufs=12)

            def named(tg):
                return thinE.tile([P, F], f32, tag="An", name=tg, bufs=12)

            _ps = {"n": 0, "banks": []}

            def psum_t(tg):
                i = _ps["n"]
                _ps["n"] += 1
                assert i < 16
                if i % 2 == 0:
                    _ps["banks"].append(
                        psth.tile([P, 2 * F], f32, tag=f"pb{i // 2}",
                                  name=f"pb{i // 2}"))
                blk = _ps["banks"][i // 2]
                off = (i % 2) * F

                class _T:
                    def __getitem__(self, _):
                        return _pl(blk, off, F)
                return _T()

            def body():
                _ps["n"] = 0
                _ps["banks"] = []
                # --------- persistent-ish tiles for this iteration
                PC = pctcp.tile([P, 3 * JF], bf16, tag="PC", name="PC")
                TC = pctcp.tile([P, 3 * JF], bf16, tag="TC", name="TC")
                means = pctcp.tile([P, 6 * F], bf16, tag="mn", name="means")
                ht = hp.tile([P, 3 * 8 * F], bf16, tag="ht", name="ht")
                H = hp.tile([P, 9 * F], f32, tag="H", name="H")
                d2 = late.tile([P, JF], bf16, tag="d2", name="d2")
                P2T2 = late.tile([P, 2 * JF], bf16, tag="p2", name="P2T2")
                Gp = late.tile([P, 9 * F], bf16, tag="G", name="Gp")

                def g3(t, off, inner=F):
                    return _ap(t, off, [[JF, 3], [F, J], [1, inner]])

                # --------- load + Act de-interleave/cast + mean + center
                for ti, (dram, ctr) in enumerate(((pred_d, PC), (targ_d, TC))):
                    for s in range(NSUB):
                        raw = rawp.tile([P, JC * SUB], f32, tag="raw",
                                        name="raw", bufs=2)
                        off = (s * SUB) * JC
                        nc.sync.dma_start(
                            raw[:], bass.AP(dram[:].tensor, off,
                                            [[F * JC, P], [1, JC * SUB]]))
                        # ctr[c][j][s*SUB+u] = raw[u*JC + j*3 + c]
                        de = E("deint", ti * NSUB + s)
                        dd_ = (_ap(ctr, s * SUB, [[JF, 3], [F, J], [1, SUB]]),
                               _ap(raw, 0, [[1, 3], [3, J], [JC, SUB]]))
                        if de is nc.scalar:
                            de.activation(dd_[0], dd_[1], AF.Copy)
                        else:
                            de.tensor_copy(dd_[0], dd_[1])
                    mn = _ap(means, ti * 3 * F, [[F, 3], [1, F]])
                    et = E("meantree", ti)
                    et.tensor_tensor(
                        _ap(ht, 0, [[8 * F, 3], [F, 8], [1, F]]),
                        _ap(ctr, 0, [[JF, 3], [F, 8], [1, F]]),
                        _ap(ctr, 8 * F, [[JF, 3], [F, 8], [1, F]]), OP.add)
                    et.tensor_tensor(
                        _ap(ht, 0, [[8 * F, 3], [F, 4], [1, F]]),
                        _ap(ht, 0, [[8 * F, 3], [F, 4], [1, F]]),
                        _ap(ht, 4 * F, [[8 * F, 3], [F, 4], [1, F]]), OP.add)
                    et.tensor_tensor(
                        _ap(ht, 0, [[8 * F, 3], [F, 2], [1, F]]),
                        _ap(ht, 0, [[8 * F, 3], [F, 2], [1, F]]),
                        _ap(ht, 2 * F, [[8 * F, 3], [F, 2], [1, F]]), OP.add)
                    et.tensor_tensor(
                        _ap(ht, 0, [[8 * F, 3], [1, F]]),
                        _ap(ht, 0, [[8 * F, 3], [1, F]]),
                        _ap(ht, F, [[8 * F, 3], [1, F]]), OP.add)
                    et.tensor_tensor(
                        mn,
                        _ap(ht, 0, [[8 * F, 3], [1, F]]),
                        _ap(ctr, 16 * F, [[JF, 3], [1, F]]), OP.add)
                    nc.vector.tensor_scalar_mul(mn, mn, 1.0 / J)
                    E("center", ti).tensor_tensor(
                        g3(ctr, 0), g3(ctr, 0),
                        _ap(means, ti * 3 * F, [[F, 3], [0, J], [1, F]]),
                        OP.subtract)

                if tap == "means":
                    nc.sync.dma_start(dbg_d[:], means[:])
                if tap == "PC":
                    nc.sync.dma_start(dbg_d[:], PC[:])
                if tap == "TC":
                    nc.sync.dma_start(dbg_d[:], TC[:])
                if stop <= 0:
                    return

                # --------- squares -> P2/T2 (Act into scratch, DVE folds)
                P2 = _pl(P2T2, 0, JF)
                T2 = _pl(P2T2, JF, JF)
                sqs = late.tile([P, JF], bf16, tag="sqs", name="sqs")
                sqh = sqs[:]
                sqd = d2[:]              # d2 free until seed
                for ti, (ctr, dst) in enumerate(((PC, P2), (TC, T2))):
                    nc.scalar.activation(dst, _pl(ctr, 0, JF), AF.Square)
                    nc.scalar.activation(sqh, _pl(ctr, JF, JF), AF.Square)
                    nc.scalar.activation(sqd, _pl(ctr, 2 * JF, JF), AF.Square)
                    E("p2fold", ti * 2).tensor_tensor(dst, dst, sqh, OP.add)
                    E("p2fold", ti * 2 + 1).tensor_tensor(dst, dst, sqd, OP.add)

                # --------- H phase + A6 interleaved
                # Op plane (r*3+c) = TC_r * PC_c ; H plane (r*3+c) = H_{c,r}
                W2 = work.tile([P, 3 * JF], bf16, tag="W2", name="W2a")

                def h_group(r):
                    E("oprod", r).tensor_tensor(
                        g3(W2, 0),
                        _ap(TC, r * JF, [[0, 3], [F, J], [1, F]]),
                        g3(PC, 0), OP.mult)
                    et = E("htree", r)
                    et.tensor_tensor(
                        _ap(ht, 0, [[8 * F, 3], [F, 8], [1, F]]),
                        _ap(W2, 0, [[JF, 3], [F, 8], [1, F]]),
                        _ap(W2, 8 * F, [[JF, 3], [F, 8], [1, F]]), OP.add)
                    et.tensor_tensor(
                        _ap(ht, 0, [[8 * F, 3], [F, 4], [1, F]]),
                        _ap(ht, 0, [[8 * F, 3], [F, 4], [1, F]]),
                        _ap(ht, 4 * F, [[8 * F, 3], [F, 4], [1, F]]), OP.add)
                    et.tensor_tensor(
                        _ap(ht, 0, [[8 * F, 3], [F, 2], [1, F]]),
                        _ap(ht, 0, [[8 * F, 3], [F, 2], [1, F]]),
                        _ap(ht, 2 * F, [[8 * F, 3], [F, 2], [1, F]]), OP.add)
                    et.tensor_tensor(
                        _ap(ht, 0, [[8 * F, 3], [1, F]]),
                        _ap(ht, 0, [[8 * F, 3], [1, F]]),
                        _ap(ht, F, [[8 * F, 3], [1, F]]), OP.add)
                    et.tensor_tensor(
                        _ap(H, r * 3 * F, [[F, 3], [1, F]]),
                        _ap(ht, 0, [[8 * F, 3], [1, F]]),
                        _ap(W2, 16 * F, [[JF, 3], [1, F]]), OP.add)

                def Hp(a, cc):
                    # H_{cc,a} (pred comp cc, targ comp a) = plane (a*3+cc)
                    return _pl(H, (a * 3 + cc) * F, F)


                A6 = {}

                def a6_entry(a, b):
                    t1 = thinE_t()
                    nc.vector.tensor_tensor(t1[:], Hp(a, 0), Hp(b, 0), OP.mult)
                    t2 = thinE_t()
                    nc.vector.tensor_tensor(t2[:], Hp(a, 1), Hp(b, 1), OP.mult)
                    nc.vector.tensor_tensor(t1[:], t1[:], t2[:], OP.add)
                    t3 = thinE_t()
                    nc.vector.tensor_tensor(t3[:], Hp(a, 2), Hp(b, 2), OP.mult)
                    At = named(f"A{a}{b}")
                    nc.vector.tensor_tensor(At[:], t1[:], t3[:], OP.add)
                    A6[(a, b)] = At

                h_group(0)
                h_group(1)
                a6_entry(0, 0)
                a6_entry(0, 1)
                a6_entry(1, 1)
                h_group(2)
                a6_entry(0, 2)
                a6_entry(1, 2)
                a6_entry(2, 2)

                # --------- sqrt -> sp2st2 (W1); pn/tn tree; s; d2 seed
                W1 = work.tile([P, 3 * JF], bf16, tag="W1", name="W1a")
                sp2st2 = _pl(W1, 0, 2 * JF)
                nc.scalar.activation(sp2st2, P2T2[:], AF.Sqrt)
                pntn = psth.tile([P, 2 * F], f32, tag="pntn", name="pntn")
                et = E("pntree")
                et.tensor_tensor(
                    _ap(ht, 0, [[8 * F, 2], [F, 8], [1, F]]),
                    _ap(W1, 0, [[JF, 2], [F, 8], [1, F]]),
                    _ap(W1, 8 * F, [[JF, 2], [F, 8], [1, F]]), OP.add)
                et.tensor_tensor(
                    _ap(ht, 0, [[8 * F, 2], [F, 4], [1, F]]),
                    _ap(ht, 0, [[8 * F, 2], [F, 4], [1, F]]),
                    _ap(ht, 4 * F, [[8 * F, 2], [F, 4], [1, F]]), OP.add)
                et.tensor_tensor(
                    _ap(ht, 0, [[8 * F, 2], [F, 2], [1, F]]),
                    _ap(ht, 0, [[8 * F, 2], [F, 2], [1, F]]),
                    _ap(ht, 2 * F, [[8 * F, 2], [F, 2], [1, F]]), OP.add)
                et.tensor_tensor(
                    _ap(ht, 0, [[8 * F, 2], [1, F]]),
                    _ap(ht, 0, [[8 * F, 2], [1, F]]),
                    _ap(ht, F, [[8 * F, 2], [1, F]]), OP.add)
                et.tensor_tensor(
                    _ap(pntn, 0, [[F, 2], [1, F]]),
                    _ap(ht, 0, [[8 * F, 2], [1, F]]),
                    _ap(W1, 16 * F, [[JF, 2], [1, F]]), OP.add)
                pn = _pl(pntn, 0, F)
                tn = _pl(pntn, F, F)
                if tap == "P2T2":
                    nc.sync.dma_start(dbg_d[:], P2T2[:])
                if tap == "pntn":
                    pncp = late.tile([P, 2 * F], f32, tag="pncp", name="pncp")
                    nc.vector.tensor_copy(pncp[:], pntn[:])
                    nc.sync.dma_start(dbg_d[:], pncp[:])

                # s = tn/(pn+eps); s2 bf16 (SBUF); seed d2 = s^2*P2 + T2
                sS = psum_t("sS")
                st_ = thinE_t()
                nc.vector.tensor_scalar_add(st_[:], pn, EPS)
                nc.vector.reciprocal_approx_fast(st_[:], st_[:])
                nc.vector.tensor_tensor(sS[:], st_[:], tn, OP.mult)
                s2b = late.tile([P, F], bf16, tag="s2b", name="s2b")
                nc.scalar.activation(s2b[:], sS[:], AF.Square)
                E("d2seed", 0).tensor_tensor(
                    d2[:], P2, _ap(s2b, 0, [[0, J], [1, F]]), OP.mult)
                E("d2seed", 1).tensor_tensor(d2[:], d2[:], T2, OP.add)

                if stop <= 2:
                    return

                a00, a01, a02 = A6[(0, 0)], A6[(0, 1)], A6[(0, 2)]
                a11, a12, a22 = A6[(1, 1)], A6[(1, 2)], A6[(2, 2)]

                # --------- eigenvalues (closed form, f32)
                q3 = thinE_t()
                nc.vector.tensor_tensor(q3[:], a00[:], a11[:], OP.add)
                nc.vector.tensor_tensor(q3[:], q3[:], a22[:], OP.add)
                m01, g0, g1 = named("m01"), named("g0"), named("g1")
                g2 = named("g2")
                nc.vector.tensor_tensor(m01[:], a01[:], a01[:], OP.mult)
                nc.vector.tensor_tensor(g0[:], a01[:], a12[:], OP.mult)
                nc.vector.tensor_tensor(g1[:], a01[:], a02[:], OP.mult)
                nc.vector.tensor_tensor(g2[:], a02[:], a12[:], OP.mult)
                m02 = thinE_t()
                nc.vector.tensor_tensor(m02[:], a02[:], a02[:], OP.mult)
                m12 = thinE_t()
                nc.vector.tensor_tensor(m12[:], a12[:], a12[:], OP.mult)
                p1 = thinE_t()
                nc.vector.tensor_tensor(p1[:], m01[:], m02[:], OP.add)
                nc.vector.tensor_tensor(p1[:], p1[:], m12[:], OP.add)
                q = named("q")
                nc.vector.tensor_scalar_mul(q[:], q3[:], 1.0 / 3)
                b00, b11, b22 = thinE_t(), thinE_t(), thinE_t()
                nc.vector.tensor_tensor(b00[:], a00[:], q[:], OP.subtract)
                nc.vector.tensor_tensor(b11[:], a11[:], q[:], OP.subtract)
                nc.vector.tensor_tensor(b22[:], a22[:], q[:], OP.subtract)
                p2s = thinE_t()
                nc.vector.tensor_tensor(p2s[:], b00[:], b00[:], OP.mult)
                tb = thinE_t()
                nc.vector.tensor_tensor(tb[:], b11[:], b11[:], OP.mult)
                nc.vector.tensor_tensor(p2s[:], p2s[:], tb[:], OP.add)
                nc.vector.tensor_tensor(tb[:], b22[:], b22[:], OP.mult)
                nc.vector.tensor_tensor(p2s[:], p2s[:], tb[:], OP.add)
                nc.vector.scalar_tensor_tensor(
                    p2s[:], p1[:], 2.0, p2s[:], OP.mult, OP.add)
                pA = named("pA")
                nc.scalar.activation(pA[:], p2s[:], AF.Sqrt, scale=1.0 / 6)
                c0 = thinE_t()
                nc.vector.tensor_tensor(c0[:], b11[:], b22[:], OP.mult)
                nc.vector.tensor_tensor(c0[:], c0[:], m12[:], OP.subtract)
                c1 = thinE_t()
                nc.vector.tensor_tensor(c1[:], a01[:], b22[:], OP.mult)
                nc.vector.tensor_tensor(c1[:], c1[:], g2[:], OP.subtract)
                c2 = thinE_t()
                nc.vector.tensor_tensor(c2[:], b11[:], a02[:], OP.mult)
                nc.vector.tensor_tensor(c2[:], g0[:], c2[:], OP.subtract)
                detB = thinE_t()
                nc.vector.tensor_tensor(detB[:], b00[:], c0[:], OP.mult)
                tdb = thinE_t()
                nc.vector.tensor_tensor(tdb[:], a01[:], c1[:], OP.mult)
                nc.vector.tensor_tensor(detB[:], detB[:], tdb[:], OP.subtract)
                nc.vector.tensor_tensor(tdb[:], a02[:], c2[:], OP.mult)
                nc.vector.tensor_tensor(detB[:], detB[:], tdb[:], OP.add)
                pinv = thinE_t()
                nc.vector.tensor_scalar_add(pinv[:], pA[:], TINY)
                nc.vector.reciprocal_approx_fast(pinv[:], pinv[:])
                p3 = thinE_t()
                nc.vector.tensor_tensor(p3[:], pinv[:], pinv[:], OP.mult)
                nc.vector.tensor_tensor(p3[:], p3[:], pinv[:], OP.mult)
                rc = thinE_t()
                nc.vector.tensor_tensor(rc[:], detB[:], p3[:], OP.mult)
                nc.vector.tensor_scalar(rc[:], rc[:], 0.5, 1.0, OP.mult, OP.min)
                nc.vector.tensor_scalar_max(rc[:], rc[:], -1.0)
                rr = thinE_t()
                nc.vector.tensor_tensor(rr[:], rc[:], rc[:], OP.mult)
                wA = thinE_t()
                nc.scalar.activation(wA[:], rr[:], AF.Sqrt, bias=1.0, scale=-1.0)
                rat = thinE_t()
                nc.vector.tensor_scalar_add(rat[:], wA[:], 1e-10)
                nc.vector.reciprocal_approx_fast(rat[:], rat[:])
                nc.vector.tensor_tensor(rat[:], rc[:], rat[:], OP.mult)
                a1 = thinE_t()
                nc.vector.tensor_scalar(a1[:], rat[:], 1.0, -1.0, OP.min, OP.max)
                rat2 = thinE_t()
                nc.vector.tensor_tensor(rat2[:], rat[:], rat[:], OP.mult)
                rinv = thinE_t()
                nc.vector.tensor_scalar_add(rinv[:], rat2[:], TINY)
                nc.vector.reciprocal_approx_fast(rinv[:], rinv[:])
                nc.vector.tensor_tensor(rinv[:], rat[:], rinv[:], OP.mult)
                nc.vector.tensor_scalar(rinv[:], rinv[:], 1.0, -1.0, OP.min, OP.max)
                sg = thinE_t()
                nc.vector.tensor_scalar(sg[:], rat[:], 1e10, 1.0, OP.mult, OP.min)
                nc.vector.tensor_scalar_max(sg[:], sg[:], -1.0)
                at1 = thinE_t()
                nc.scalar.activation(at1[:], a1[:], AF.Arctan)
                at2 = thinE_t()
                nc.scalar.activation(at2[:], rinv[:], AF.Arctan)
                atb = thinE_t()
                nc.vector.scalar_tensor_tensor(
                    atb[:], sg[:], 1.5707963267948966, at2[:],
                    OP.mult, OP.subtract)
                m_ = thinE_t()
                nc.vector.tensor_scalar_add(m_[:], rat2[:], -1.0)
                nc.vector.tensor_scalar(m_[:], m_[:], 1e10, 1.0, OP.mult, OP.min)
                nc.vector.tensor_scalar_max(m_[:], m_[:], 0.0)
                atn = thinE_t()
                nc.vector.tensor_tensor(atn[:], atb[:], at1[:], OP.subtract)
                nc.vector.tensor_tensor(atn[:], atn[:], m_[:], OP.mult)
                nc.vector.tensor_tensor(atn[:], atn[:], at1[:], OP.add)
                cs1 = psum_t("cs1")
                nc.scalar.activation(cs1[:], atn[:], AF.Sin,
                                     bias=b2p3[:], scale=-1.0 / 3)
                cs2 = psum_t("cs2")
                nc.scalar.activation(cs2[:], atn[:], AF.Sin,
                                     bias=b4p3[:], scale=-1.0 / 3)
                lam0, lam1 = psum_t("lam0"), psum_t("lam1")
                tp = thinE_t()
                nc.vector.tensor_tensor(tp[:], pA[:], cs1[:], OP.mult)
                nc.vector.scalar_tensor_tensor(
                    lam0[:], tp[:], 2.0, q[:], OP.mult, OP.add)
                lam2 = thinE_t()
                nc.vector.tensor_tensor(tp[:], pA[:], cs2[:], OP.mult)
                nc.vector.scalar_tensor_tensor(
                    lam2[:], tp[:], -2.0, q[:], OP.mult, OP.add)
                nc.vector.scalar_tensor_tensor(
                    lam1[:], q[:], 3.0, lam0[:], OP.mult, OP.subtract)
                nc.vector.tensor_tensor(lam1[:], lam1[:], lam2[:], OP.subtract)

                # --------- W1 carve for bf16 tail (sp2st2 dead after
                # pntree). layout: Hb 9F | vb 9F | ub 6F | u2t 3F | gt 3F |
                # gt2 3F | rsb 2F | invsb F
                W1b = work.tile([P, 3 * JF], bf16, tag="W1", name="W1b")
                invsb = _pl(W1b, 35 * F, F)
                nc.vector.tensor_copy(_pl(W1b, 0, 9 * F), H[:])

                def vbp(i, k):
                    return _pl(W1b, (9 + i * 3 + k) * F, F)

                # --------- eigenvectors v0, v1 (f32 transient -> bf16 vb)
                def eigvec(lam, vbi):
                    vx = thinE_t()
                    vy = thinE_t()
                    vz = thinE_t()
                    b0 = thinE_t()
                    nc.vector.tensor_tensor(b0[:], a00[:], lam[:], OP.subtract)
                    b1 = thinE_t()
                    nc.vector.tensor_tensor(b1[:], a11[:], lam[:], OP.subtract)
                    nc.vector.tensor_tensor(vx[:], a02[:], b1[:], OP.mult)
                    nc.vector.tensor_tensor(vx[:], g0[:], vx[:], OP.subtract)
                    nc.vector.tensor_tensor(vy[:], b0[:], a12[:], OP.mult)
                    nc.vector.tensor_tensor(vy[:], g1[:], vy[:], OP.subtract)
                    nc.vector.tensor_tensor(vz[:], b0[:], b1[:], OP.mult)
                    nc.vector.tensor_tensor(vz[:], vz[:], m01[:], OP.subtract)
                    n2 = thinE_t()
                    nc.vector.tensor_tensor(n2[:], vx[:], vx[:], OP.mult)
                    t2_ = thinE_t()
                    nc.vector.tensor_tensor(t2_[:], vy[:], vy[:], OP.mult)
                    nc.vector.tensor_tensor(n2[:], n2[:], t2_[:], OP.add)
                    nc.vector.tensor_tensor(t2_[:], vz[:], vz[:], OP.mult)
                    nc.vector.tensor_tensor(n2[:], n2[:], t2_[:], OP.add)
                    ns = thinE_t()
                    nc.scalar.activation(ns[:], n2[:], AF.Sqrt)
                    nc.vector.tensor_scalar_add(ns[:], ns[:], TINY)
                    nc.vector.reciprocal_approx_fast(ns[:], ns[:])
                    nc.vector.tensor_tensor(vbp(vbi, 0), vx[:], ns[:], OP.mult)
                    nc.vector.tensor_tensor(vbp(vbi, 1), vy[:], ns[:], OP.mult)
                    nc.vector.tensor_tensor(vbp(vbi, 2), vz[:], ns[:], OP.mult)

                eigvec(lam0, 0)
                eigvec(lam1, 1)

                # v2 = v0 x v1 (bf16)
                cr = ((1, 2), (2, 0), (0, 1))
                for r_ in range(3):
                    i1, i2 = cr[r_]
                    t1b = _pl(W1b, 27 * F, F)
                    t2b = _pl(W1b, 28 * F, F)
                    nc.vector.tensor_tensor(t1b, vbp(0, i1), vbp(1, i2),
                                            OP.mult)
                    nc.vector.tensor_tensor(t2b, vbp(0, i2), vbp(1, i1),
                                            OP.mult)
                    nc.vector.tensor_tensor(vbp(2, r_), t1b, t2b, OP.subtract)

                # --------- rsig_i = -2s/sigma_i (bf16 into rsb)
                for i, lam in enumerate((lam0, lam1)):
                    rl = thinE_t()
                    nc.scalar.activation(rl[:], lam[:], AF.Relu)
                    sg_ = thinE_t()
                    nc.scalar.activation(sg_[:], rl[:], AF.Sqrt)
                    nc.vector.tensor_scalar_add(sg_[:], sg_[:], TINY)
                    nc.vector.reciprocal_approx_fast(sg_[:], sg_[:])
                    nc.vector.scalar_tensor_tensor(
                        _pl(W1b, (33 + i) * F, F), sg_[:], -2.0, sS[:],
                        OP.mult, OP.mult)
                iv_ = thinE_t()
                nc.vector.tensor_scalar_add(iv_[:], sS[:], TINY)
                nc.vector.reciprocal_approx_fast(iv_[:], iv_[:])
                nc.vector.tensor_scalar_mul(iv_[:], iv_[:], -0.5)
                nc.vector.tensor_copy(invsb, iv_[:])

                def HCg(k):
                    # planes (k*3 + r) = H_{r,k}, r=0..2
                    return _ap(W1b, k * 3 * F, [[F, 3], [1, F]])

                def vbc(i, k):
                    return _ap(W1b, (9 + i * 3 + k) * F, [[0, 3], [1, F]])

                # u_i[r] = sum_k H_{r,k} (v_i)_k, scaled by rsig_i
                for i in range(2):
                    udst = _ap(W1b, (18 + i * 3) * F, [[F, 3], [1, F]])
                    nc.vector.tensor_tensor(udst, HCg(0), vbc(i, 0), OP.mult)
                    gta = _ap(W1b, 27 * F, [[F, 3], [1, F]])
                    nc.vector.tensor_tensor(gta, HCg(1), vbc(i, 1), OP.mult)
                    nc.vector.tensor_tensor(udst, udst, gta, OP.add)
                    nc.vector.tensor_tensor(gta, HCg(2), vbc(i, 2), OP.mult)
                    nc.vector.tensor_tensor(udst, udst, gta, OP.add)
                    nc.vector.tensor_tensor(
                        udst, udst, _ap(W1b, (33 + i) * F, [[0, 3], [1, F]]),
                        OP.mult)

                def up(ui, r_):
                    return _pl(W1b, (18 + ui * 3 + r_) * F, F)

                # u2 = cross(u0, u1) * (-0.5/s)
                for r_ in range(3):
                    i1, i2 = cr[r_]
                    t1b = _pl(W1b, 27 * F, F)
                    t2b = _pl(W1b, 28 * F, F)
                    nc.vector.tensor_tensor(t1b, up(0, i1), up(1, i2), OP.mult)
                    nc.vector.tensor_tensor(t2b, up(0, i2), up(1, i1), OP.mult)
                    nc.vector.tensor_tensor(t1b, t1b, t2b, OP.subtract)
                    nc.vector.tensor_tensor(
                        _pl(W1b, (24 + r_) * F, F), t1b, invsb, OP.mult)

                # --------- G' plane (r*3+c) = sum_i u_i[c] * (v_i)_r
                def ug(i):
                    base = (18 + i * 3) * F if i < 2 else 24 * F
                    return _ap(W1b, base, [[F, 3], [1, F]])

                gta = _ap(W1b, 27 * F, [[F, 3], [1, F]])
                gtb = _ap(W1b, 30 * F, [[F, 3], [1, F]])
                for r_ in range(3):
                    # G' plane (r*3+c) = sum_i u_i[c] * v_r[i]  (V^T quirk of
                    # the reference: R = Vh @ Ut, so the contraction pairs
                    # u_i with the i-th COMPONENT of v_r)
                    Grg = _ap(Gp, r_ * 3 * F, [[F, 3], [1, F]])
                    nc.vector.tensor_tensor(gta, ug(0), vbc(r_, 0), OP.mult)
                    nc.vector.tensor_tensor(gtb, ug(1), vbc(r_, 1), OP.mult)
                    nc.vector.tensor_tensor(gta, gta, gtb, OP.add)
                    nc.vector.tensor_tensor(gtb, ug(2), vbc(r_, 2), OP.mult)
                    nc.vector.tensor_tensor(Grg, gta, gtb, OP.add)

                if tap == "G":
                    nc.sync.dma_start(dbg_d[:], Gp[:])
                if stop <= 3:
                    return

                # --------- e-phase: E_acc[r] = sum_c PC_c * G'_{c,r}
                W2b = work.tile([P, 3 * JF], bf16, tag="W2", name="W2b")
                Ea = g3(W2b, 0)
                W1c = work.tile([P, 3 * JF], bf16, tag="W1", name="W1c")
                Et = g3(W1c, 0)

                def gpc(c):
                    # G' planes (r*3+c) for r=0..2: offset c*F, stride 3F
                    return _ap(Gp, c * F, [[3 * F, 3], [0, J], [1, F]])

                E("eprod", 0).tensor_tensor(
                    Ea, _ap(PC, 0, [[0, 3], [F, J], [1, F]]), gpc(0), OP.mult)
                E("eprod", 1).tensor_tensor(
                    Et, _ap(PC, JF, [[0, 3], [F, J], [1, F]]), gpc(1), OP.mult)
                E("eacc", 0).tensor_tensor(Ea, Ea, Et, OP.add)
                E("eprod", 2).tensor_tensor(
                    Et, _ap(PC, 2 * JF, [[0, 3], [F, J], [1, F]]), gpc(2),
                    OP.mult)
                E("eacc", 1).tensor_tensor(Ea, Ea, Et, OP.add)
                # Et = E_acc * TC (aligned r-planes); fold into d2
                E("emul").tensor_tensor(Et, Ea, g3(TC, 0), OP.mult)
                for c in range(3):
                    E("wd2", c).tensor_tensor(
                        d2[:], d2[:], _pl(W1c, c * JF, JF), OP.add)

                if tap == "d2":
                    nc.sync.dma_start(dbg_d[:], d2[:])
                # --------- dist = sqrt(max(d2,0)); j-tree; accumulate
                E("d2max").tensor_scalar_max(d2[:], d2[:], 0.0)
                dr = sqs[:]
                nc.scalar.activation(dr, d2[:], AF.Sqrt)
                dh = Gp  # dist-tree scratch aliases G (dead after e-prods)
                et = E("dsum")
                et.tensor_tensor(
                    _ap(dh, 0, [[F, 8], [1, F]]),
                    _ap(sqs, 0, [[F, 8], [1, F]]),
                    _ap(sqs, 8 * F, [[F, 8], [1, F]]), OP.add)
                et.tensor_tensor(
                    _ap(dh, 0, [[F, 4], [1, F]]),
                    _ap(dh, 0, [[F, 4], [1, F]]),
                    _ap(dh, 4 * F, [[F, 4], [1, F]]), OP.add)
                et.tensor_tensor(
                    _ap(dh, 0, [[F, 2], [1, F]]),
                    _ap(dh, 0, [[F, 2], [1, F]]),
                    _ap(dh, 2 * F, [[F, 2], [1, F]]), OP.add)
                et.tensor_tensor(
                    _pl(dh, 0, F), _pl(dh, 0, F), _pl(dh, F, F), OP.add)
                et.tensor_tensor(
                    _pl(dh, 0, F), _pl(dh, 0, F), _pl(sqs, 16 * F, F), OP.add)
                nc.vector.tensor_tensor(acc[:], acc[:], _pl(dh, 0, F), OP.add)

            if iters == unroll or iters == 1:
                for _ in range(max(iters, 1) if iters == unroll else 1):
                    body()
            elif unroll > 1 and iters % unroll == 0:
                # multiple bodies per trip: the all-engine barrier For_i
                # inserts per trip amortizes, and body k+1's front overlaps
                # body k's tail through the tag-ring dependencies.
                with tc.For_i(0, iters // unroll, 1):
                    for _ in range(unroll):
                        body()
            else:
                with tc.For_i(0, iters, 1):
                    body()

            accs = persist.tile([P, 1], f32, tag="accs", name="accs")
            nc.vector.tensor_reduce(accs[:], acc[:], axis=AX.X, op=OP.add)
            nc.sync.dma_start(out_d[:], accs[:])

    nc.compile()
    return nc


def build_tapped(tap):
    nc = build_nc(iters=1, tap=tap)
    return nc, (lambda x: x)


_nc_cache = None


def get_nc():
    global _nc_cache
    if _nc_cache is None:
        _nc_cache = build_nc()
    return _nc_cache


def run(nc, pred, target, trace=False, **kw):
    pred2 = np.ascontiguousarray(np.asarray(pred), np.float32).reshape(B, JC)
    targ2 = np.ascontiguousarray(np.asarray(target), np.float32).reshape(B, JC)
    in_maps = [
        {"pred": pred2[c * BC:(c + 1) * BC], "target": targ2[c * BC:(c + 1) * BC]}
        for c in range(NCORES)
    ]
    res = run_bass_kernel_spmd(nc, in_maps, list(range(NCORES)), trace=trace, **kw)
    total = sum(r["partial"].astype(np.float64).sum() for r in res.results)
    loss = np.float32(total / (B * J))
    return loss, res


def kernel(pred, target):
    loss, _ = run(get_nc(), pred, target)
    return loss


# revision 28
# speedup vs baseline: 1.5068x; 1.0995x over previous
"""Batched Procrustes-alignment loss on 8 Trainium2 NeuronCores.

Data-parallel over batch (B=262144 -> 32768/core), laid out as [128
partitions, F=256] planes (one scalar per batch element per plane).

v2 pipeline (per core, per For_i iteration):
  DMA raw [P, 51*SUB] f32 sub-chunks; Act de-interleaves+casts to bf16
  component planes [P, 3*JF] (PC/TC after in-place centering). DVE fused-3
  bf16 j-trees give means; fused-3 in-place centered subtract.
  Squares via Act into small ping-pong scratch; folds -> P2/T2 bf16;
  Act sqrt -> sp2st2 (work slot W1); fused-2 j-tree -> pn/tn (PSUM);
  s = tn/(pn+eps) early; d2 = s^2*P2 + T2 seeded before the SVD tail.
  H phase: per r-group one fused-3 product TC_r*PC into work slot W2 and
  a fused-3 bf16 j-tree -> H[r*3+c] = H_{c,r} (f32).
  Closed-form 3x3 eigensolver on A = H^T H (A6 in PSUM, trig eigenvalues,
  eigvecs via cross-of-rows, u_i = H v_i * (-2s/sigma_i), u2 = cross/-2s);
  G' plane (r*3+c) = sum_i u_i[c] v_i[r] (-2s folded), assembled in bf16
  carved out of W1.
  e-phase (no stored O): E_acc[r] = sum_c PC_c * G'_{c,r} (bcast over j),
  then Et = E_acc * TC, folded into d2. dist = sqrt(max(d2,0)); j-tree
  sum; acc += dsum. Host sums [P,1] partials in float64.
"""
import numpy as np
import concourse.bass as bass
import concourse.mybir as mybir
import concourse.tile as tile
from concourse import bacc
from concourse.bass_utils import run_bass_kernel_spmd

AF = mybir.ActivationFunctionType
OP = mybir.AluOpType
AX = mybir.AxisListType
f32 = mybir.dt.float32
bf16 = mybir.dt.bfloat16

B, J, C = 262144, 17, 3
JC = J * C
NCORES = 8
BC = B // NCORES
P = 128
F = 256
JF = J * F
SUB = 64
NSUB = F // SUB
EPS = 1e-8
TINY = 1e-20

# engine assignment knobs ("v" = DVE vector, "g" = gpsimd Pool, "s" = scalar/Act)
KNOBS = dict(
    deint=["s"] * 8,          # per (tensor*NSUB + sub)
    meantree=["v", "v"],      # per tensor
    center=["v", "v"],        # per tensor
    p2fold=["v", "v", "v", "v"],   # P2+=sq1, P2+=sq2, T2+=sq1, T2+=sq2
    pntree="v",
    d2seed=["v", "v"],        # d2 = P2*s2, d2 += T2
    oprod=["v", "v", "v"],    # per r
    htree=["v", "v", "v"],    # per r
    eprod=["v", "v", "v"],    # per c
    eacc=["v", "v"],
    emul="v",
    wd2=["v", "v", "v"],      # three JF folds into d2
    d2max="v",
    dsum="v",
)


def _ap(t, off, dims):
    a = t[:]
    return bass.AP(a.tensor, a.offset + off, [a.ap[0]] + dims)


def _pl(t, off, n):
    return _ap(t, off, [[1, n]])


def build_nc(iters=1, knobs=None, stop=99, tap=None, unroll=4):
    kn = dict(KNOBS)
    if knobs:
        kn.update(knobs)
    TAP_SHAPES = dict(means=6 * F, PC=3 * JF, TC=3 * JF, P2T2=2 * JF,
                      pntn=2 * F, H=9 * F, G=9 * F, d2=JF)

    nc = bacc.Bacc("TRN2", target_bir_lowering=False)
    pred_d = nc.dram_tensor("pred", [BC, JC], f32, kind="ExternalInput")
    targ_d = nc.dram_tensor("target", [BC, JC], f32, kind="ExternalInput")
    out_d = nc.dram_tensor("partial", [P, 1], f32, kind="ExternalOutput")
    dbg_d = (nc.dram_tensor("dbg", [P, TAP_SHAPES[tap]],
                            f32 if tap in ("H", "pntn") else bf16,
                            kind="ExternalOutput") if tap else None)

    def E(key, i=None):
        v = kn[key] if i is None else kn[key][i]
        return {"v": nc.vector, "g": nc.gpsimd, "s": nc.scalar}[v]

    with tile.TileContext(nc) as tc:
        with (
            tc.tile_pool(name="persist", bufs=1) as persist,
            tc.tile_pool(name="rawp", bufs=1) as rawp,
            tc.tile_pool(name="pctc", bufs=1) as pctcp,
            tc.tile_pool(name="work", bufs=1) as work,
            tc.tile_pool(name="hp", bufs=1) as hp,
            tc.tile_pool(name="late", bufs=1) as late,
            tc.tile_pool(name="thinE", bufs=1) as thinE,
            tc.tile_pool(name="psth", bufs=1, space="PSUM") as psth,
        ):
            acc = persist.tile([P, F], f32, tag="acc", name="acc")
            b2p3 = persist.tile([P, 1], f32, tag="b2p3", name="b2p3")
            b4p3 = persist.tile([P, 1], f32, tag="b4p3", name="b4p3")
            nc.gpsimd.memset(acc[:], 0.0)
            nc.gpsimd.memset(b2p3[:], 2.0943951023931953)  # 2pi/3
            nc.gpsimd.memset(b4p3[:], 1.0471975511965976)  # pi/3

            def thinE_t():
                return thinE.tile([P, F], f32, tag="te", name="te", bufs=12)

            def named(tg):
                return thinE.tile([P, F], f32, tag="An", name=tg, bufs=12)

            _ps = {"n": 0, "banks": []}

            def psum_t(tg):
                i = _ps["n"]
                _ps["n"] += 1
                assert i < 16
                if i % 2 == 0:
                    _ps["banks"].append(
                        psth.tile([P, 2 * F], f32, tag=f"pb{i // 2}",
                                  name=f"pb{i // 2}"))
                blk = _ps["banks"][i // 2]
                off = (i % 2) * F

                class _T:
                    def __getitem__(self, _):
                        return _pl(blk, off, F)
                return _T()

            def body():
                _ps["n"] = 0
                _ps["banks"] = []
                # --------- persistent-ish tiles for this iteration
                PC = pctcp.tile([P, 3 * JF], bf16, tag="PC", name="PC")
                TC = pctcp.tile([P, 3 * JF], bf16, tag="TC", name="TC")
                means = pctcp.tile([P, 6 * F], bf16, tag="mn", name="means")
                ht = hp.tile([P, 3 * 8 * F], bf16, tag="ht", name="ht")
                H = hp.tile([P, 9 * F], f32, tag="H", name="H")
                d2 = late.tile([P, JF], bf16, tag="d2", name="d2")
                P2T2 = late.tile([P, 2 * JF], bf16, tag="p2", name="P2T2")
                Gp = late.tile([P, 9 * F], bf16, tag="G", name="Gp")

                def g3(t, off, inner=F):
                    return _ap(t, off, [[JF, 3], [F, J], [1, inner]])

                # --------- load + Act de-interleave/cast + mean + center
                for ti, (dram, ctr) in enumerate(((pred_d, PC), (targ_d, TC))):
                    for s in range(NSUB):
                        raw = rawp.tile([P, JC * SUB], f32, tag="raw",
                                        name="raw", bufs=2)
                        off = (s * SUB) * JC
                        nc.sync.dma_start(
                            raw[:], bass.AP(dram[:].tensor, off,
                                            [[F * JC, P], [1, JC * SUB]]))
                        # ctr[c][j][s*SUB+u] = raw[u*JC + j*3 + c]
                        de = E("deint", ti * NSUB + s)
                        dd_ = (_ap(ctr, s * SUB, [[JF, 3], [F, J], [1, SUB]]),
                               _ap(raw, 0, [[1, 3], [3, J], [JC, SUB]]))
                        if de is nc.scalar:
                            de.activation(dd_[0], dd_[1], AF.Copy)
                        else:
                            de.tensor_copy(dd_[0], dd_[1])
                    mn = _ap(means, ti * 3 * F, [[F, 3], [1, F]])
                    et = E("meantree", ti)
                    et.tensor_tensor(
                        _ap(ht, 0, [[8 * F, 3], [F, 8], [1, F]]),
                        _ap(ctr, 0, [[JF, 3], [F, 8], [1, F]]),
                        _ap(ctr, 8 * F, [[JF, 3], [F, 8], [1, F]]), OP.add)
                    et.tensor_tensor(
                        _ap(ht, 0, [[8 * F, 3], [F, 4], [1, F]]),
                        _ap(ht, 0, [[8 * F, 3], [F, 4], [1, F]]),
                        _ap(ht, 4 * F, [[8 * F, 3], [F, 4], [1, F]]), OP.add)
                    et.tensor_tensor(
                        _ap(ht, 0, [[8 * F, 3], [F, 2], [1, F]]),
                        _ap(ht, 0, [[8 * F, 3], [F, 2], [1, F]]),
                        _ap(ht, 2 * F, [[8 * F, 3], [F, 2], [1, F]]), OP.add)
                    et.tensor_tensor(
                        _ap(ht, 0, [[8 * F, 3], [1, F]]),
                        _ap(ht, 0, [[8 * F, 3], [1, F]]),
                        _ap(ht, F, [[8 * F, 3], [1, F]]), OP.add)
                    et.tensor_tensor(
                        mn,
                        _ap(ht, 0, [[8 * F, 3], [1, F]]),
                        _ap(ctr, 16 * F, [[JF, 3], [1, F]]), OP.add)
                    nc.vector.tensor_scalar_mul(mn, mn, 1.0 / J)
                    E("center", ti).tensor_tensor(
                        g3(ctr, 0), g3(ctr, 0),
                        _ap(means, ti * 3 * F, [[F, 3], [0, J], [1, F]]),
                        OP.subtract)

                if tap == "means":
                    nc.sync.dma_start(dbg_d[:], means[:])
                if tap == "PC":
                    nc.sync.dma_start(dbg_d[:], PC[:])
                if tap == "TC":
                    nc.sync.dma_start(dbg_d[:], TC[:])
                if stop <= 0:
                    return

                # --------- squares -> P2/T2 (Act into scratch, DVE folds)
                P2 = _pl(P2T2, 0, JF)
                T2 = _pl(P2T2, JF, JF)
                # W1a hosts: squares scratch (3rd JF plane), then sp2st2
                # (planes 0-1) for the pn/tn tree
                W1 = work.tile([P, 3 * JF], bf16, tag="W1", name="W1a")
                sqh = _pl(W1, 2 * JF, JF)
                sqd = d2[:]              # d2 free until seed
                for ti, (ctr, dst) in enumerate(((PC, P2), (TC, T2))):
                    nc.scalar.activation(dst, _pl(ctr, 0, JF), AF.Square)
                    nc.scalar.activation(sqh, _pl(ctr, JF, JF), AF.Square)
                    nc.scalar.activation(sqd, _pl(ctr, 2 * JF, JF), AF.Square)
                    E("p2fold", ti * 2).tensor_tensor(dst, dst, sqh, OP.add)
                    E("p2fold", ti * 2 + 1).tensor_tensor(dst, dst, sqd, OP.add)

                # --------- H phase + A6 interleaved
                # Op plane (r*3+c) = TC_r * PC_c ; H plane (r*3+c) = H_{c,r}
                W2 = work.tile([P, 3 * JF], bf16, tag="W2", name="W2a")

                def h_group(r):
                    E("oprod", r).tensor_tensor(
                        g3(W2, 0),
                        _ap(TC, r * JF, [[0, 3], [F, J], [1, F]]),
                        g3(PC, 0), OP.mult)
                    et = E("htree", r)
                    et.tensor_tensor(
                        _ap(ht, 0, [[8 * F, 3], [F, 8], [1, F]]),
                        _ap(W2, 0, [[JF, 3], [F, 8], [1, F]]),
                        _ap(W2, 8 * F, [[JF, 3], [F, 8], [1, F]]), OP.add)
                    et.tensor_tensor(
                        _ap(ht, 0, [[8 * F, 3], [F, 4], [1, F]]),
                        _ap(ht, 0, [[8 * F, 3], [F, 4], [1, F]]),
                        _ap(ht, 4 * F, [[8 * F, 3], [F, 4], [1, F]]), OP.add)
                    et.tensor_tensor(
                        _ap(ht, 0, [[8 * F, 3], [F, 2], [1, F]]),
                        _ap(ht, 0, [[8 * F, 3], [F, 2], [1, F]]),
                        _ap(ht, 2 * F, [[8 * F, 3], [F, 2], [1, F]]), OP.add)
                    et.tensor_tensor(
                        _ap(ht, 0, [[8 * F, 3], [1, F]]),
                        _ap(ht, 0, [[8 * F, 3], [1, F]]),
                        _ap(ht, F, [[8 * F, 3], [1, F]]), OP.add)
                    et.tensor_tensor(
                        _ap(H, r * 3 * F, [[F, 3], [1, F]]),
                        _ap(ht, 0, [[8 * F, 3], [1, F]]),
                        _ap(W2, 16 * F, [[JF, 3], [1, F]]), OP.add)

                def Hp(a, cc):
                    # H_{cc,a} (pred comp cc, targ comp a) = plane (a*3+cc)
                    return _pl(H, (a * 3 + cc) * F, F)


                A6 = {}

                def a6_entry(a, b):
                    t1 = thinE_t()
                    nc.vector.tensor_tensor(t1[:], Hp(a, 0), Hp(b, 0), OP.mult)
                    t2 = thinE_t()
                    nc.vector.tensor_tensor(t2[:], Hp(a, 1), Hp(b, 1), OP.mult)
                    nc.vector.tensor_tensor(t1[:], t1[:], t2[:], OP.add)
                    t3 = thinE_t()
                    nc.vector.tensor_tensor(t3[:], Hp(a, 2), Hp(b, 2), OP.mult)
                    At = named(f"A{a}{b}")
                    nc.vector.tensor_tensor(At[:], t1[:], t3[:], OP.add)
                    A6[(a, b)] = At

                h_group(0)
                h_group(1)
                a6_entry(0, 0)
                a6_entry(0, 1)
                a6_entry(1, 1)
                h_group(2)
                a6_entry(0, 2)
                a6_entry(1, 2)
                a6_entry(2, 2)

                # --------- sqrt -> sp2st2 (W1 planes 0-1); pn/tn tree; s
                sp2st2 = _pl(W1, 0, 2 * JF)
                nc.scalar.activation(sp2st2, P2T2[:], AF.Sqrt)
                pntn = psth.tile([P, 2 * F], f32, tag="pntn", name="pntn")
                et = E("pntree")
                et.tensor_tensor(
                    _ap(ht, 0, [[8 * F, 2], [F, 8], [1, F]]),
                    _ap(W1, 0, [[JF, 2], [F, 8], [1, F]]),
                    _ap(W1, 8 * F, [[JF, 2], [F, 8], [1, F]]), OP.add)
                et.tensor_tensor(
                    _ap(ht, 0, [[8 * F, 2], [F, 4], [1, F]]),
                    _ap(ht, 0, [[8 * F, 2], [F, 4], [1, F]]),
                    _ap(ht, 4 * F, [[8 * F, 2], [F, 4], [1, F]]), OP.add)
                et.tensor_tensor(
                    _ap(ht, 0, [[8 * F, 2], [F, 2], [1, F]]),
                    _ap(ht, 0, [[8 * F, 2], [F, 2], [1, F]]),
                    _ap(ht, 2 * F, [[8 * F, 2], [F, 2], [1, F]]), OP.add)
                et.tensor_tensor(
                    _ap(ht, 0, [[8 * F, 2], [1, F]]),
                    _ap(ht, 0, [[8 * F, 2], [1, F]]),
                    _ap(ht, F, [[8 * F, 2], [1, F]]), OP.add)
                et.tensor_tensor(
                    _ap(pntn, 0, [[F, 2], [1, F]]),
                    _ap(ht, 0, [[8 * F, 2], [1, F]]),
                    _ap(W1, 16 * F, [[JF, 2], [1, F]]), OP.add)
                pn = _pl(pntn, 0, F)
                tn = _pl(pntn, F, F)
                if tap == "P2T2":
                    nc.sync.dma_start(dbg_d[:], P2T2[:])
                if tap == "pntn":
                    pncp = late.tile([P, 2 * F], f32, tag="pncp", name="pncp")
                    nc.vector.tensor_copy(pncp[:], pntn[:])
                    nc.sync.dma_start(dbg_d[:], pncp[:])

                # s = tn/(pn+eps); s2 bf16 (SBUF); seed d2 = s^2*P2 + T2
                sS = psum_t("sS")
                st_ = thinE_t()
                nc.vector.tensor_scalar_add(st_[:], pn, EPS)
                nc.vector.reciprocal_approx_fast(st_[:], st_[:])
                nc.vector.tensor_tensor(sS[:], st_[:], tn, OP.mult)
                s2b = late.tile([P, F], bf16, tag="s2b", name="s2b")
                nc.scalar.activation(s2b[:], sS[:], AF.Square)
                E("d2seed", 0).tensor_tensor(
                    d2[:], P2, _ap(s2b, 0, [[0, J], [1, F]]), OP.mult)
                E("d2seed", 1).tensor_tensor(d2[:], d2[:], T2, OP.add)

                if stop <= 2:
                    return

                a00, a01, a02 = A6[(0, 0)], A6[(0, 1)], A6[(0, 2)]
                a11, a12, a22 = A6[(1, 1)], A6[(1, 2)], A6[(2, 2)]

                # --------- eigenvalues (closed form, f32)
                q3 = thinE_t()
                nc.vector.tensor_tensor(q3[:], a00[:], a11[:], OP.add)
                nc.vector.tensor_tensor(q3[:], q3[:], a22[:], OP.add)
                m01, g0, g1 = named("m01"), named("g0"), named("g1")
                g2 = named("g2")
                nc.vector.tensor_tensor(m01[:], a01[:], a01[:], OP.mult)
                nc.vector.tensor_tensor(g0[:], a01[:], a12[:], OP.mult)
                nc.vector.tensor_tensor(g1[:], a01[:], a02[:], OP.mult)
                nc.vector.tensor_tensor(g2[:], a02[:], a12[:], OP.mult)
                m02 = thinE_t()
                nc.vector.tensor_tensor(m02[:], a02[:], a02[:], OP.mult)
                m12 = thinE_t()
                nc.vector.tensor_tensor(m12[:], a12[:], a12[:], OP.mult)
                p1 = thinE_t()
                nc.vector.tensor_tensor(p1[:], m01[:], m02[:], OP.add)
                nc.vector.tensor_tensor(p1[:], p1[:], m12[:], OP.add)
                q = named("q")
                nc.vector.tensor_scalar_mul(q[:], q3[:], 1.0 / 3)
                b00, b11, b22 = thinE_t(), thinE_t(), thinE_t()
                nc.vector.tensor_tensor(b00[:], a00[:], q[:], OP.subtract)
                nc.vector.tensor_tensor(b11[:], a11[:], q[:], OP.subtract)
                nc.vector.tensor_tensor(b22[:], a22[:], q[:], OP.subtract)
                p2s = thinE_t()
                nc.vector.tensor_tensor(p2s[:], b00[:], b00[:], OP.mult)
                tb = thinE_t()
                nc.vector.tensor_tensor(tb[:], b11[:], b11[:], OP.mult)
                nc.vector.tensor_tensor(p2s[:], p2s[:], tb[:], OP.add)
                nc.vector.tensor_tensor(tb[:], b22[:], b22[:], OP.mult)
                nc.vector.tensor_tensor(p2s[:], p2s[:], tb[:], OP.add)
                nc.vector.scalar_tensor_tensor(
                    p2s[:], p1[:], 2.0, p2s[:], OP.mult, OP.add)
                pA = named("pA")
                nc.scalar.activation(pA[:], p2s[:], AF.Sqrt, scale=1.0 / 6)
                c0 = thinE_t()
                nc.vector.tensor_tensor(c0[:], b11[:], b22[:], OP.mult)
                nc.vector.tensor_tensor(c0[:], c0[:], m12[:], OP.subtract)
                c1 = thinE_t()
                nc.vector.tensor_tensor(c1[:], a01[:], b22[:], OP.mult)
                nc.vector.tensor_tensor(c1[:], c1[:], g2[:], OP.subtract)
                c2 = thinE_t()
                nc.vector.tensor_tensor(c2[:], b11[:], a02[:], OP.mult)
                nc.vector.tensor_tensor(c2[:], g0[:], c2[:], OP.subtract)
                detB = thinE_t()
                nc.vector.tensor_tensor(detB[:], b00[:], c0[:], OP.mult)
                tdb = thinE_t()
                nc.vector.tensor_tensor(tdb[:], a01[:], c1[:], OP.mult)
                nc.vector.tensor_tensor(detB[:], detB[:], tdb[:], OP.subtract)
                nc.vector.tensor_tensor(tdb[:], a02[:], c2[:], OP.mult)
                nc.vector.tensor_tensor(detB[:], detB[:], tdb[:], OP.add)
                pinv = thinE_t()
                nc.vector.tensor_scalar_add(pinv[:], pA[:], TINY)
                nc.vector.reciprocal_approx_fast(pinv[:], pinv[:])
                p3 = thinE_t()
                nc.vector.tensor_tensor(p3[:], pinv[:], pinv[:], OP.mult)
                nc.vector.tensor_tensor(p3[:], p3[:], pinv[:], OP.mult)
                rc = thinE_t()
                nc.vector.tensor_tensor(rc[:], detB[:], p3[:], OP.mult)
                nc.vector.tensor_scalar(rc[:], rc[:], 0.5, 1.0, OP.mult, OP.min)
                nc.vector.tensor_scalar_max(rc[:], rc[:], -1.0)
                rr = thinE_t()
                nc.vector.tensor_tensor(rr[:], rc[:], rc[:], OP.mult)
                wA = thinE_t()
                nc.scalar.activation(wA[:], rr[:], AF.Sqrt, bias=1.0, scale=-1.0)
                rat = thinE_t()
                nc.vector.tensor_scalar_add(rat[:], wA[:], 1e-10)
                nc.vector.reciprocal_approx_fast(rat[:], rat[:])
                nc.vector.tensor_tensor(rat[:], rc[:], rat[:], OP.mult)
                a1 = thinE_t()
                nc.vector.tensor_scalar(a1[:], rat[:], 1.0, -1.0, OP.min, OP.max)
                rat2 = thinE_t()
                nc.vector.tensor_tensor(rat2[:], rat[:], rat[:], OP.mult)
                rinv = thinE_t()
                nc.vector.tensor_scalar_add(rinv[:], rat2[:], TINY)
                nc.vector.reciprocal_approx_fast(rinv[:], rinv[:])
                nc.vector.tensor_tensor(rinv[:], rat[:], rinv[:], OP.mult)
                nc.vector.tensor_scalar(rinv[:], rinv[:], 1.0, -1.0, OP.min, OP.max)
                sg = thinE_t()
                nc.vector.tensor_scalar(sg[:], rat[:], 1e10, 1.0, OP.mult, OP.min)
                nc.vector.tensor_scalar_max(sg[:], sg[:], -1.0)
                at1 = thinE_t()
                nc.scalar.activation(at1[:], a1[:], AF.Arctan)
                at2 = thinE_t()
                nc.scalar.activation(at2[:], rinv[:], AF.Arctan)
                atb = thinE_t()
                nc.vector.scalar_tensor_tensor(
                    atb[:], sg[:], 1.5707963267948966, at2[:],
                    OP.mult, OP.subtract)
                m_ = thinE_t()
                nc.vector.tensor_scalar_add(m_[:], rat2[:], -1.0)
                nc.vector.tensor_scalar(m_[:], m_[:], 1e10, 1.0, OP.mult, OP.min)
                nc.vector.tensor_scalar_max(m_[:], m_[:], 0.0)
                atn = thinE_t()
                nc.vector.tensor_tensor(atn[:], atb[:], at1[:], OP.subtract)
                nc.vector.tensor_tensor(atn[:], atn[:], m_[:], OP.mult)
                nc.vector.tensor_tensor(atn[:], atn[:], at1[:], OP.add)
                cs1 = psum_t("cs1")
                nc.scalar.activation(cs1[:], atn[:], AF.Sin,
                                     bias=b2p3[:], scale=-1.0 / 3)
                cs2 = psum_t("cs2")
                nc.scalar.activation(cs2[:], atn[:], AF.Sin,
                                     bias=b4p3[:], scale=-1.0 / 3)
                lam0, lam1 = psum_t("lam0"), psum_t("lam1")
                tp = thinE_t()
                nc.vector.tensor_tensor(tp[:], pA[:], cs1[:], OP.mult)
                nc.vector.scalar_tensor_tensor(
                    lam0[:], tp[:], 2.0, q[:], OP.mult, OP.add)
                lam2 = thinE_t()
                nc.vector.tensor_tensor(tp[:], pA[:], cs2[:], OP.mult)
                nc.vector.scalar_tensor_tensor(
                    lam2[:], tp[:], -2.0, q[:], OP.mult, OP.add)
                nc.vector.scalar_tensor_tensor(
                    lam1[:], q[:], 3.0, lam0[:], OP.mult, OP.subtract)
                nc.vector.tensor_tensor(lam1[:], lam1[:], lam2[:], OP.subtract)

                # --------- W1 carve for bf16 tail (sp2st2 dead after
                # pntree). layout: Hb 9F | vb 9F | ub 6F | u2t 3F | gt 3F |
                # gt2 3F | rsb 2F | invsb F
                W1b = work.tile([P, 3 * JF], bf16, tag="W1", name="W1b")
                invsb = _pl(W1b, 35 * F, F)
                nc.vector.tensor_copy(_pl(W1b, 0, 9 * F), H[:])

                def vbp(i, k):
                    return _pl(W1b, (9 + i * 3 + k) * F, F)

                # --------- eigenvectors v0, v1 (f32 transient -> bf16 vb)
                def eigvec(lam, vbi):
                    vx = thinE_t()
                    vy = thinE_t()
                    vz = thinE_t()
                    b0 = thinE_t()
                    nc.vector.tensor_tensor(b0[:], a00[:], lam[:], OP.subtract)
                    b1 = thinE_t()
                    nc.vector.tensor_tensor(b1[:], a11[:], lam[:], OP.subtract)
                    nc.vector.tensor_tensor(vx[:], a02[:], b1[:], OP.mult)
                    nc.vector.tensor_tensor(vx[:], g0[:], vx[:], OP.subtract)
                    nc.vector.tensor_tensor(vy[:], b0[:], a12[:], OP.mult)
                    nc.vector.tensor_tensor(vy[:], g1[:], vy[:], OP.subtract)
                    nc.vector.tensor_tensor(vz[:], b0[:], b1[:], OP.mult)
                    nc.vector.tensor_tensor(vz[:], vz[:], m01[:], OP.subtract)
                    n2 = thinE_t()
                    nc.vector.tensor_tensor(n2[:], vx[:], vx[:], OP.mult)
                    t2_ = thinE_t()
                    nc.vector.tensor_tensor(t2_[:], vy[:], vy[:], OP.mult)
                    nc.vector.tensor_tensor(n2[:], n2[:], t2_[:], OP.add)
                    nc.vector.tensor_tensor(t2_[:], vz[:], vz[:], OP.mult)
                    nc.vector.tensor_tensor(n2[:], n2[:], t2_[:], OP.add)
                    ns = thinE_t()
                    nc.scalar.activation(ns[:], n2[:], AF.Sqrt)
                    nc.vector.tensor_scalar_add(ns[:], ns[:], TINY)
                    nc.vector.reciprocal_approx_fast(ns[:], ns[:])
                    nc.vector.tensor_tensor(vbp(vbi, 0), vx[:], ns[:], OP.mult)
                    nc.vector.tensor_tensor(vbp(vbi, 1), vy[:], ns[:], OP.mult)
                    nc.vector.tensor_tensor(vbp(vbi, 2), vz[:], ns[:], OP.mult)

                eigvec(lam0, 0)
                eigvec(lam1, 1)

                # v2 = v0 x v1 (bf16)
                cr = ((1, 2), (2, 0), (0, 1))
                for r_ in range(3):
                    i1, i2 = cr[r_]
                    t1b = _pl(W1b, 27 * F, F)
                    t2b = _pl(W1b, 28 * F, F)
                    nc.vector.tensor_tensor(t1b, vbp(0, i1), vbp(1, i2),
                                            OP.mult)
                    nc.vector.tensor_tensor(t2b, vbp(0, i2), vbp(1, i1),
                                            OP.mult)
                    nc.vector.tensor_tensor(vbp(2, r_), t1b, t2b, OP.subtract)

                # --------- rsig_i = -2s/sigma_i (bf16 into rsb)
                for i, lam in enumerate((lam0, lam1)):
                    rl = thinE_t()
                    nc.scalar.activation(rl[:], lam[:], AF.Relu)
                    sg_ = thinE_t()
                    nc.scalar.activation(sg_[:], rl[:], AF.Sqrt)
                    nc.vector.tensor_scalar_add(sg_[:], sg_[:], TINY)
                    nc.vector.reciprocal_approx_fast(sg_[:], sg_[:])
                    nc.vector.scalar_tensor_tensor(
                        _pl(W1b, (33 + i) * F, F), sg_[:], -2.0, sS[:],
                        OP.mult, OP.mult)
                iv_ = thinE_t()
                nc.vector.tensor_scalar_add(iv_[:], sS[:], TINY)
                nc.vector.reciprocal_approx_fast(iv_[:], iv_[:])
                nc.vector.tensor_scalar_mul(iv_[:], iv_[:], -0.5)
                nc.vector.tensor_copy(invsb, iv_[:])

                def HCg(k):
                    # planes (k*3 + r) = H_{r,k}, r=0..2
                    return _ap(W1b, k * 3 * F, [[F, 3], [1, F]])

                def vbc(i, k):
                    return _ap(W1b, (9 + i * 3 + k) * F, [[0, 3], [1, F]])

                # u_i[r] = sum_k H_{r,k} (v_i)_k, scaled by rsig_i
                for i in range(2):
                    udst = _ap(W1b, (18 + i * 3) * F, [[F, 3], [1, F]])
                    nc.vector.tensor_tensor(udst, HCg(0), vbc(i, 0), OP.mult)
                    gta = _ap(W1b, 27 * F, [[F, 3], [1, F]])
                    nc.vector.tensor_tensor(gta, HCg(1), vbc(i, 1), OP.mult)
                    nc.vector.tensor_tensor(udst, udst, gta, OP.add)
                    nc.vector.tensor_tensor(gta, HCg(2), vbc(i, 2), OP.mult)
                    nc.vector.tensor_tensor(udst, udst, gta, OP.add)
                    nc.vector.tensor_tensor(
                        udst, udst, _ap(W1b, (33 + i) * F, [[0, 3], [1, F]]),
                        OP.mult)

                def up(ui, r_):
                    return _pl(W1b, (18 + ui * 3 + r_) * F, F)

                # u2 = cross(u0, u1) * (-0.5/s)
                for r_ in range(3):
                    i1, i2 = cr[r_]
                    t1b = _pl(W1b, 27 * F, F)
                    t2b = _pl(W1b, 28 * F, F)
                    nc.vector.tensor_tensor(t1b, up(0, i1), up(1, i2), OP.mult)
                    nc.vector.tensor_tensor(t2b, up(0, i2), up(1, i1), OP.mult)
                    nc.vector.tensor_tensor(t1b, t1b, t2b, OP.subtract)
                    nc.vector.tensor_tensor(
                        _pl(W1b, (24 + r_) * F, F), t1b, invsb, OP.mult)

                # --------- G' plane (r*3+c) = sum_i u_i[c] * (v_i)_r
                def ug(i):
                    base = (18 + i * 3) * F if i < 2 else 24 * F
                    return _ap(W1b, base, [[F, 3], [1, F]])

                gta = _ap(W1b, 27 * F, [[F, 3], [1, F]])
                gtb = _ap(W1b, 30 * F, [[F, 3], [1, F]])
                for r_ in range(3):
                    # G' plane (r*3+c) = sum_i u_i[c] * v_r[i]  (V^T quirk of
                    # the reference: R = Vh @ Ut, so the contraction pairs
                    # u_i with the i-th COMPONENT of v_r)
                    Grg = _ap(Gp, r_ * 3 * F, [[F, 3], [1, F]])
                    nc.vector.tensor_tensor(gta, ug(0), vbc(r_, 0), OP.mult)
                    nc.vector.tensor_tensor(gtb, ug(1), vbc(r_, 1), OP.mult)
                    nc.vector.tensor_tensor(gta, gta, gtb, OP.add)
                    nc.vector.tensor_tensor(gtb, ug(2), vbc(r_, 2), OP.mult)
                    nc.vector.tensor_tensor(Grg, gta, gtb, OP.add)

                if tap == "G":
                    nc.sync.dma_start(dbg_d[:], Gp[:])
                if stop <= 3:
                    return

                # --------- e-phase: E_acc[r] = sum_c PC_c * G'_{c,r}
                W2b = work.tile([P, 3 * JF], bf16, tag="W2", name="W2b")
                Ea = g3(W2b, 0)
                W1c = work.tile([P, 3 * JF], bf16, tag="W1", name="W1c")
                Et = g3(W1c, 0)

                def gpc(c):
                    # G' planes (r*3+c) for r=0..2: offset c*F, stride 3F
                    return _ap(Gp, c * F, [[3 * F, 3], [0, J], [1, F]])

                E("eprod", 0).tensor_tensor(
                    Ea, _ap(PC, 0, [[0, 3], [F, J], [1, F]]), gpc(0), OP.mult)
                E("eprod", 1).tensor_tensor(
                    Et, _ap(PC, JF, [[0, 3], [F, J], [1, F]]), gpc(1), OP.mult)
                E("eacc", 0).tensor_tensor(Ea, Ea, Et, OP.add)
                E("eprod", 2).tensor_tensor(
                    Et, _ap(PC, 2 * JF, [[0, 3], [F, J], [1, F]]), gpc(2),
                    OP.mult)
                E("eacc", 1).tensor_tensor(Ea, Ea, Et, OP.add)
                # Et = E_acc * TC (aligned r-planes); fold into d2
                E("emul").tensor_tensor(Et, Ea, g3(TC, 0), OP.mult)
                for c in range(3):
                    E("wd2", c).tensor_tensor(
                        d2[:], d2[:], _pl(W1c, c * JF, JF), OP.add)

                if tap == "d2":
                    nc.sync.dma_start(dbg_d[:], d2[:])
                # --------- dist = sqrt(max(d2,0)); j-tree; accumulate
                E("d2max").tensor_scalar_max(d2[:], d2[:], 0.0)
                dr = _pl(W1c, 0, JF)     # Et dead after wd2
                nc.scalar.activation(dr, d2[:], AF.Sqrt)
                dh = Gp  # dist-tree scratch aliases G (dead after e-prods)
                et = E("dsum")
                et.tensor_tensor(
                    _ap(dh, 0, [[F, 8], [1, F]]),
                    _ap(W1c, 0, [[F, 8], [1, F]]),
                    _ap(W1c, 8 * F, [[F, 8], [1, F]]), OP.add)
                et.tensor_tensor(
                    _ap(dh, 0, [[F, 4], [1, F]]),
                    _ap(dh, 0, [[F, 4], [1, F]]),
                    _ap(dh, 4 * F, [[F, 4], [1, F]]), OP.add)
                et.tensor_tensor(
                    _ap(dh, 0, [[F, 2], [1, F]]),
                    _ap(dh, 0, [[F, 2], [1, F]]),
                    _ap(dh, 2 * F, [[F, 2], [1, F]]), OP.add)
                et.tensor_tensor(
                    _pl(dh, 0, F), _pl(dh, 0, F), _pl(dh, F, F), OP.add)
                et.tensor_tensor(
                    _pl(dh, 0, F), _pl(dh, 0, F), _pl(W1c, 16 * F, F), OP.add)
                nc.vector.tensor_tensor(acc[:], acc[:], _pl(dh, 0, F), OP.add)

            if iters == unroll or iters == 1:
                for _ in range(max(iters, 1) if iters == unroll else 1):
                    body()
            elif unroll > 1 and iters % unroll == 0:
                # multiple bodies per trip: the all-engine barrier For_i
                # inserts per trip amortizes, and body k+1's front overlaps
                # body k's tail through the tag-ring dependencies.
                with tc.For_i(0, iters // unroll, 1):
                    for _ in range(unroll):
                        body()
            else:
                with tc.For_i(0, iters, 1):
                    body()

            accs = persist.tile([P, 1], f32, tag="accs", name="accs")
            nc.vector.tensor_reduce(accs[:], acc[:], axis=AX.X, op=OP.add)
            nc.sync.dma_start(out_d[:], accs[:])

    nc.compile()
    return nc


def build_tapped(tap):
    nc = build_nc(iters=1, tap=tap)
    return nc, (lambda x: x)


_nc_cache = None


def get_nc():
    global _nc_cache
    if _nc_cache is None:
        _nc_cache = build_nc()
    return _nc_cache


def run(nc, pred, target, trace=False, **kw):
    pred2 = np.ascontiguousarray(np.asarray(pred), np.float32).reshape(B, JC)
    targ2 = np.ascontiguousarray(np.asarray(target), np.float32).reshape(B, JC)
    in_maps = [
        {"pred": pred2[c * BC:(c + 1) * BC], "target": targ2[c * BC:(c + 1) * BC]}
        for c in range(NCORES)
    ]
    res = run_bass_kernel_spmd(nc, in_maps, list(range(NCORES)), trace=trace, **kw)
    total = sum(r["partial"].astype(np.float64).sum() for r in res.results)
    loss = np.float32(total / (B * J))
    return loss, res


def kernel(pred, target):
    loss, _ = run(get_nc(), pred, target)
    return loss
